# revision 26
# baseline (speedup 1.0000x reference)
"""BinaryLeNet5 forward on 8 TRN2 NeuronCores, pure data parallel (1024 imgs/core).

Mapping summary (per core):
  conv1: kh-accumulated banded-Toeplitz matmuls. lhsT T1[kh] = [96=(c,wi), 168=(mc,wop,wo2,o)]
         built host-side from binarized w1; rhs = direct row-slices of transposed input
         xt[(c,w), (h,n)]. conv1 touches real-valued x, and the net is chaotic in the
         conv1 signs (even 2^-17 rounding flips enough signs to fail the 2e-2 gate), so
         the rhs is an fp16 hi+lo split (residual 2^-22 -> measured 0 final error),
         two accumulating passes per kh at 1 cycle/row each (fp32 native is 4 cyc/row).
  pool+sign: maxpool ho-pairs are the two contiguous halves of each PSUM tile and
         wo-pairs are the two wo-parity PSUM tiles -> 3 contiguous DVE tensor_max ops
         (strided reduce_max was ~1.8x slower), then ACT Sign with f32 per-partition
         bias (bias exact; commutes with max).
  conv2: inputs/weights exactly +-1 -> fp8e4 exact; kh pairs {0,1},{2,3} run as
         DoubleRow matmuls (2 fp8 weights/PE cell, K_eff=168, ~2x column rate), kh4 as
         a plain fp8 matmul. f32 PSUM accumulation of +-1 sums is exact.
  fc1/fc2/fc3: +-1 bf16 matmuls, f32 PSUM, biases applied in f32 via ACT bias operand.
  hardtanh drops out everywhere: sign(clip(x)) == sign(x), max(clip) == clip(max).

DMA: two HW DGE issue queues (sync=hi tiles, scalar=lo tiles+weights); input split
into 2-h-row tiles so the first matmul waits on ~0.55MB, not the full 12.6MB.

Output written as [10, 1024] per core, transposed/stacked on host.
"""

import os
import sys

import numpy as np

sys.path.insert(0, "/opt/trn_rl_repo")

import ml_dtypes  # noqa: E402

BF16 = ml_dtypes.bfloat16
F8E4 = ml_dtypes.float8_e4m3

B = 8192
NCORES = 8
N = B // NCORES  # 1024 images per core
NBLK = 2  # n blocks of 512 columns
NB = N // NBLK  # 512

CONV2_DR = int(os.environ.get("CONV2_DR", "1"))  # fp8 DoubleRow conv2
# conv1 lo-pass mode: "dr8" = both fp8 lo-terms in one DoubleRow matmul
# (descale 2^-14 folded into e5m2 weights; rel err ~0.011, deterministic),
# "f16" = single fp16 lo pass (exact).
CONV1_LO = os.environ.get("CONV1_LO", "dr8")
LOSC = 14  # lo-term scale exponent


def _binarize(w):
    return np.where(w >= 0, 1.0, -1.0).astype(np.float32)


def _build_t1(w1):
    # t1[c*32+wi, kh*168 + par*84 + wo2*6 + o] = w1b[o,c,kh,kw]
    #   wo = 2*wo2 + par (par = wo parity), kw = wi - wo, valid 0<=kw<5
    # Chunk A (par=0) = all even wo, chunk B = all odd wo -> the 2x2 maxpool's
    # wo-pair max is a same-partition tensor_max of the two chunk results.
    w1b = _binarize(w1)  # [6,3,5,5]
    t1 = np.zeros((96, 5 * 168), np.float32)
    for kh in range(5):
        for par in range(2):
            for wo2 in range(14):
                wo = 2 * wo2 + par
                for o in range(6):
                    col = kh * 168 + par * 84 + wo2 * 6 + o
                    for c in range(3):
                        for kw in range(5):
                            wi = wo + kw
                            if wi < 32:
                                t1[c * 32 + wi, col] = w1b[o, c, kh, kw]
    return t1


def _build_t2(w2):
    # t2[w2*6+c, kh*160 + wop*80 + wo2*16 + o] = w2b[o,c,kh,kw], kw = w2-(2*wo2+wop)
    w2b = _binarize(w2)  # [16,6,5,5]
    t2 = np.zeros((84, 5 * 160), np.float32)
    for kh in range(5):
        for wop in range(2):
            for wo2 in range(5):
                wo = 2 * wo2 + wop
                for o in range(16):
                    col = kh * 160 + wop * 80 + wo2 * 16 + o
                    for c in range(6):
                        for kw in range(5):
                            w2i = wo + kw
                            if w2i < 14:
                                t2[w2i * 6 + c, col] = w2b[o, c, kh, kw]
    return t2


def _build_f1(wf1):
    # f1[w*16+o, h*120+f] = wf1b[f, o*25+h*5+w]
    wf1b = _binarize(wf1)  # [120, 400]
    f1 = np.zeros((80, 5 * 120), np.float32)
    for h in range(5):
        for w in range(5):
            for o in range(16):
                f1[w * 16 + o, h * 120 : (h + 1) * 120] = wf1b[:, o * 25 + h * 5 + w]
    return f1


_CACHE = {}


def _get_nc(dr, lo):
    key = f"nc_dr{dr}_{lo}"
    if key in _CACHE:
        return _CACHE[key]
    import concourse.bacc as bacc
    import concourse.mybir as mybir
    import concourse.tile as tile

    f32 = mybir.dt.float32
    f16 = mybir.dt.float16
    bf16 = mybir.dt.bfloat16
    f8e4 = mybir.dt.float8e4
    f8e5 = mybir.dt.float8e5
    x2dt = f8e4 if dr else bf16

    nc = bacc.Bacc()
    xh_d = nc.dram_tensor("xh", [96, 32 * N], f16, kind="ExternalInput")
    if lo == "dr8":
        # per 2-row tile k: [l1 row0 | l1 row1 | l2 row0 | l2 row1], N cols each
        xlo_d = nc.dram_tensor("xlo", [96, 64 * N], f8e4, kind="ExternalInput")
        # per (kh,par): [i0: 96 cols | i1: 96 cols], both = t1 slice * 2^-LOSC,
        # M padded 84->96 (DR weight step must be %16==0)
        t1lo_d = nc.dram_tensor("t1lo", [96, 1920], f8e5, kind="ExternalInput")
    else:
        xl_d = nc.dram_tensor("xl", [96, 32 * N], f16, kind="ExternalInput")
    t1_d = nc.dram_tensor("t1", [96, 840], f16, kind="ExternalInput")
    if dr:
        t2a_d = nc.dram_tensor("t2a", [84, 640], f8e4, kind="ExternalInput")
        t2b_d = nc.dram_tensor("t2b", [84, 160], f8e4, kind="ExternalInput")
        f1d_d = nc.dram_tensor("f1d", [80, 512], f8e4, kind="ExternalInput")
        f14_d = nc.dram_tensor("f14", [80, 128], f8e4, kind="ExternalInput")
    else:
        t2_d = nc.dram_tensor("t2", [84, 800], bf16, kind="ExternalInput")
        f1_d = nc.dram_tensor("f1", [80, 600], bf16, kind="ExternalInput")
    f2_d = nc.dram_tensor("f2", [120, 84], bf16, kind="ExternalInput")
    f3_d = nc.dram_tensor("f3", [84, 10], bf16, kind="ExternalInput")
    b1_d = nc.dram_tensor("b1v", [84, 1], f32, kind="ExternalInput")
    b2_d = nc.dram_tensor("b2v", [80, 1], f32, kind="ExternalInput")
    bf1_d = nc.dram_tensor("bf1v", [120, 1], f32, kind="ExternalInput")
    bf2_d = nc.dram_tensor("bf2v", [84, 1], f32, kind="ExternalInput")
    bf3_d = nc.dram_tensor("bf3v", [10, 1], f32, kind="ExternalInput")
    out_d = nc.dram_tensor("out", [10, N], f32, kind="ExternalOutput")

    HCH = 2  # h rows per xt sbuf tile
    NT = 32 // HCH  # 16 tiles per hi/lo
    with tile.TileContext(nc) as tc:
        with (
            tc.tile_pool(name="xtp", bufs=1) as xtp,
            tc.tile_pool(name="wts", bufs=1) as wts,
            tc.tile_pool(name="acts", bufs=1) as acts,
            tc.tile_pool(name="ev", bufs=3) as ev,
            tc.tile_pool(name="ps", bufs=4, space="PSUM") as ps,
        ):
            # ---- DMA plan: sync queue = t1 + hi tiles (conv1 critical path);
            # scalar queue = first lo tiles, then the conv2/fc weights+biases
            # (needed from ~30us on), then the remaining lo tiles. ----
            t1s = wts.tile([96, 840], f16, tag="t1")
            nc.scalar.dma_start(out=t1s, in_=t1_d[:, :])
            if lo == "dr8":
                t1los = wts.tile([96, 1920], f8e5, tag="t1lo")
                nc.scalar.dma_start(out=t1los, in_=t1lo_d[:, :])

            LCH = 2 * HCH * N if lo == "dr8" else HCH * N  # lo cols per tile
            lodt = f8e4 if lo == "dr8" else f16
            lod = xlo_d if lo == "dr8" else xl_d
            xhs, xls = [], []
            for k in range(NT):
                th = xtp.tile([96, HCH * N], f16, tag=f"xh{k}", name=f"xh{k}")
                nc.sync.dma_start(out=th, in_=xh_d[:, k * HCH * N : (k + 1) * HCH * N])
                xhs.append(th)
            for k in range(3):
                tl = xtp.tile([96, LCH], lodt, tag=f"xl{k}", name=f"xl{k}")
                nc.scalar.dma_start(out=tl, in_=lod[:, k * LCH : (k + 1) * LCH])
                xls.append(tl)

            if dr:
                t2as = wts.tile([84, 640], f8e4, tag="t2a")
                nc.scalar.dma_start(out=t2as, in_=t2a_d[:, :])
                t2bs = wts.tile([84, 160], f8e4, tag="t2b")
                nc.scalar.dma_start(out=t2bs, in_=t2b_d[:, :])
                f1ds = wts.tile([80, 512], f8e4, tag="f1d")
                nc.scalar.dma_start(out=f1ds, in_=f1d_d[:, :])
                f14s = wts.tile([80, 128], f8e4, tag="f14")
                nc.scalar.dma_start(out=f14s, in_=f14_d[:, :])
            else:
                t2s = wts.tile([84, 800], bf16, tag="t2")
                nc.scalar.dma_start(out=t2s, in_=t2_d[:, :])
                f1s = wts.tile([80, 600], bf16, tag="f1")
                nc.scalar.dma_start(out=f1s, in_=f1_d[:, :])
            f2s = wts.tile([120, 84], bf16, tag="f2")
            nc.scalar.dma_start(out=f2s, in_=f2_d[:, :])
            f3s = wts.tile([84, 10], bf16, tag="f3")
            nc.scalar.dma_start(out=f3s, in_=f3_d[:, :])
            b1s = wts.tile([84, 1], f32, tag="b1")
            nc.scalar.dma_start(out=b1s, in_=b1_d[:, :])
            b2s = wts.tile([80, 1], f32, tag="b2")
            nc.scalar.dma_start(out=b2s, in_=b2_d[:, :])
            bf1s = wts.tile([120, 1], f32, tag="bf1")
            nc.scalar.dma_start(out=bf1s, in_=bf1_d[:, :])
            bf2s = wts.tile([84, 1], f32, tag="bf2")
            nc.scalar.dma_start(out=bf2s, in_=bf2_d[:, :])
            bf3s = wts.tile([10, 1], f32, tag="bf3")
            nc.scalar.dma_start(out=bf3s, in_=bf3_d[:, :])

            for k in range(3, NT):
                tl = xtp.tile([96, LCH], lodt, tag=f"xl{k}", name=f"xl{k}")
                nc.scalar.dma_start(out=tl, in_=lod[:, k * LCH : (k + 1) * LCH])
                xls.append(tl)

            def xhi(h, nb):  # hi rhs [96, NB] for input row h
                off = (h % HCH) * N + nb * NB
                return xhs[h // HCH][:, off : off + NB]

            def xlo(h, nb):
                if lo == "dr8":  # [96, 2, NB]: (l1, l2) pair for DoubleRow
                    t = xls[h // HCH].rearrange("p (i c) -> p i c", i=2)
                    off = (h % HCH) * N + nb * NB
                    return t[:, :, off : off + NB]
                off = (h % HCH) * N + nb * NB
                return xls[h // HCH][:, off : off + NB]

            # One consumer-engine 'touch' per DMA'd bias tile: the touch op
            # carries the DMA wait, so later ops on that engine need no extra
            # wait slot (TRN2 engine instructions have a single wait slot).
            tb1 = wts.tile([84, 1], f32, tag="tb1")
            nc.scalar.copy(tb1, b1s)
            tb2 = wts.tile([80, 1], f32, tag="tb2")
            nc.scalar.copy(tb2, b2s)
            tb3 = wts.tile([120, 1], f32, tag="tb3")
            nc.scalar.copy(tb3, bf1s)
            tb4 = wts.tile([84, 1], f32, tag="tb4")
            nc.scalar.copy(tb4, bf2s)
            tb5 = wts.tile([10, 1], f32, tag="tb5")
            nc.vector.tensor_copy(tb5, bf3s)

            x2 = acts.tile([84, 14 * N], x2dt, tag="x2")
            x3 = acts.tile([80, 5 * N], f8e4 if dr else bf16, tag="x3")
            x4 = acts.tile([120, N], bf16, tag="x4")
            x5 = acts.tile([84, N], bf16, tag="x5")
            outs = acts.tile([10, N], f32, tag="outs")

            # ---- conv1 + pool + sign -> x2 ----
            # psum tile [84, 1024] = (hop 2) x (n 512) blocks; ho-pair pooled as
            # tensor_max of the tile's two contiguous halves; wo-pair = tensor_max
            # of the two parity tiles (same partitions). No strided DVE reads.
            PM = 96 if lo == "dr8" else 84  # psum partitions (DR pads M 84->96)
            for ho2 in range(14):
                for nb in range(NBLK):
                    p = [ps.tile([PM, 2 * NB], f32, tag="ps", name="p1") for _ in range(2)]
                    for par in range(2):
                        for kh in range(5):
                            lhs = t1s[:, kh * 168 + par * 84 : kh * 168 + par * 84 + 84]
                            for hop in range(2):
                                h = 2 * ho2 + hop + kh
                                out96 = p[par][:, hop * NB : hop * NB + NB]
                                out84 = p[par][0:84, hop * NB : hop * NB + NB]
                                if lo == "dr8":
                                    # DR lo covers all 96 rows: it opens (kh0) and
                                    # closes (kh4) the accumulation; hi (84 rows)
                                    # accumulates in between.
                                    lod_lhs = t1los[
                                        :, (kh * 2 + par) * 192 : (kh * 2 + par) * 192 + 192
                                    ].rearrange("p (i m) -> p i m", i=2)
                                    if kh < 4:
                                        nc.tensor.matmul(
                                            out96, lod_lhs, xlo(h, nb),
                                            start=(kh == 0), stop=False,
                                            perf_mode=mybir.MatmulPerfMode.DoubleRow,
                                        )
                                        nc.tensor.matmul(
                                            out84, lhs, xhi(h, nb),
                                            start=False, stop=False,
                                            skip_group_check=True,
                                        )
                                    else:
                                        nc.tensor.matmul(
                                            out84, lhs, xhi(h, nb),
                                            start=False, stop=False,
                                            skip_group_check=True,
                                        )
                                        nc.tensor.matmul(
                                            out96, lod_lhs, xlo(h, nb),
                                            start=False, stop=True,
                                            perf_mode=mybir.MatmulPerfMode.DoubleRow,
                                        )
                                else:
                                    nc.tensor.matmul(
                                        out84, lhs, xhi(h, nb),
                                        start=(kh == 0), stop=False,
                                    )
                                    nc.tensor.matmul(
                                        out84, lhs, xlo(h, nb),
                                        start=False, stop=(kh == 4),
                                    )
                    e1 = []
                    for par in range(2):
                        e = ev.tile([96, NB], f32, tag="ea", name="e1")[0:84]
                        nc.vector.reduce_max(e, p[par][0:84, :].rearrange("q (h n) -> q n h", h=2), axis=mybir.AxisListType.X)
                        e1.append(e)
                    e2 = ev.tile([96, NB], f32, tag="ec", name="e2")[0:84]
                    nc.vector.tensor_max(e2, e1[0], e1[1])
                    nc.scalar.sign(
                        x2[:, ho2 * N + nb * NB : ho2 * N + nb * NB + NB],
                        e2, bias=b1s,
                    )
            # ---- conv2 + pool + sign -> x3 ----
            x2r = x2.rearrange("p (h n) -> p h n", h=14)
            for ho2 in range(5):
                for nb in range(NBLK):
                    p2 = [ps.tile([80, 2 * NB], f32, tag="ps", name="p2") for _ in range(2)]
                    for wop in range(2):
                        if dr:
                            for pair in range(2):  # kh {0,1} and {2,3}
                                lhs = t2as[
                                    :, pair * 320 + wop * 160 : pair * 320 + wop * 160 + 160
                                ].rearrange("p (i m) -> p i m", i=2)
                                for hop in range(2):
                                    h0 = 2 * ho2 + hop + 2 * pair
                                    nc.tensor.matmul(
                                        p2[wop][:, hop * NB : hop * NB + NB], lhs,
                                        x2r[:, h0 : h0 + 2, nb * NB : nb * NB + NB],
                                        start=(pair == 0), stop=False,
                                        perf_mode=mybir.MatmulPerfMode.DoubleRow,
                                    )
                            lhs4 = t2bs[:, wop * 80 : wop * 80 + 80]
                            for hop in range(2):
                                h_in = 2 * ho2 + hop + 4
                                nc.tensor.matmul(
                                    p2[wop][:, hop * NB : hop * NB + NB], lhs4,
                                    x2[:, h_in * N + nb * NB : h_in * N + nb * NB + NB],
                                    start=False, stop=True,
                                )
                        else:
                            for kh in range(5):
                                lhs = t2s[:, kh * 160 + wop * 80 : kh * 160 + wop * 80 + 80]
                                for hop in range(2):
                                    h_in = 2 * ho2 + hop + kh
                                    nc.tensor.matmul(
                                        p2[wop][:, hop * NB : hop * NB + NB], lhs,
                                        x2[:, h_in * N + nb * NB : h_in * N + nb * NB + NB],
                                        start=(kh == 0), stop=(kh == 4),
                                    )
                    ew = []
                    for wop in range(2):
                        # conv2 PSUM values are exact small integers -> bf16 pooled
                        # tiles are exact and 16-bit DVE ops run at 2x rate.
                        e = ev.tile([96, NB], bf16, tag="eb", name="e3")[0:80]
                        nc.vector.reduce_max(e, p2[wop].rearrange("q (h n) -> q n h", h=2), axis=mybir.AxisListType.X)
                        ew.append(e)
                    e4 = ev.tile([96, NB], bf16, tag="ed", name="e4")[0:80]
                    nc.vector.tensor_max(e4, ew[0], ew[1])
                    nc.scalar.sign(
                        x3[:, ho2 * N + nb * NB : ho2 * N + nb * NB + NB], e4, bias=b2s
                    )

            # ---- fc1 -> fc2 -> fc3 ----
            # 4 n-sub-blocks of 256 with stage-interleaving: shortens the exposed
            # serial MM->sign->MM chain at the kernel tail; fc1 h5-pairs {0,1},{2,3}
            # run as fp8 DoubleRow (M padded 120->128), h5=4 as a plain fp8 matmul.
            NBF = 2
            NF = N // NBF  # 512
            x3r = x3.rearrange("p (h n) -> p h n", h=5)
            p3 = [
                ps.tile([128 if dr else 120, NF], f32, tag="ps", name="p3")
                for _ in range(NBF)
            ]
            for nb in range(NBF):
                if dr:
                    for pair in range(2):
                        lhs = f1ds[:, pair * 256 : pair * 256 + 256].rearrange(
                            "p (i m) -> p i m", i=2
                        )
                        nc.tensor.matmul(
                            p3[nb], lhs,
                            x3r[:, 2 * pair : 2 * pair + 2, nb * NF : nb * NF + NF],
                            start=(pair == 0), stop=False,
                            perf_mode=mybir.MatmulPerfMode.DoubleRow,
                        )
                    nc.tensor.matmul(
                        p3[nb], f14s,
                        x3[:, 4 * N + nb * NF : 4 * N + nb * NF + NF],
                        start=False, stop=True,
                    )
                else:
                    for h5 in range(5):
                        nc.tensor.matmul(
                            p3[nb], f1s[:, h5 * 120 : (h5 + 1) * 120],
                            x3[:, h5 * N + nb * NF : h5 * N + nb * NF + NF],
                            start=(h5 == 0), stop=(h5 == 4),
                        )
            for nb in range(NBF):
                nc.scalar.sign(x4[:, nb * NF : nb * NF + NF], p3[nb][0:120], bias=bf1s)
            p4 = [ps.tile([84, NF], f32, tag="ps", name="p4") for _ in range(NBF)]
            for nb in range(NBF):
                nc.tensor.matmul(p4[nb], f2s, x4[:, nb * NF : nb * NF + NF], start=True, stop=True)
            for nb in range(NBF):
                nc.scalar.sign(x5[:, nb * NF : nb * NF + NF], p4[nb], bias=bf2s)
            p5 = [ps.tile([10, NF], f32, tag="ps", name="p5") for _ in range(NBF)]
            for nb in range(NBF):
                nc.tensor.matmul(p5[nb], f3s, x5[:, nb * NF : nb * NF + NF], start=True, stop=True)
            for nb in range(NBF):
                nc.vector.tensor_scalar_add(outs[:, nb * NF : nb * NF + NF], p5[nb], bf3s)
                nc.sync.dma_start(
                    out=out_d[:, nb * NF : nb * NF + NF],
                    in_=outs[:, nb * NF : nb * NF + NF],
                )

    nc.finalize()
    _CACHE[key] = nc
    return nc


def _install_ntff_hook():
    """The container's antenv stub lacks axon_hooks; synthesize it and register
    the ctypes-based NTFF profile hook from the axon boot module."""
    if "hook" in _CACHE:
        return
    _CACHE["hook"] = True
    try:
        import types
        import antenv

        if not hasattr(antenv, "axon_hooks"):
            store = {"h": None}
            m = types.ModuleType("antenv.axon_hooks")
            m.set_axon_ntff_profile_hook = lambda h: store.update(h=h)
            m.get_axon_ntff_profile_hook = lambda: store["h"]
            sys.modules["antenv.axon_hooks"] = m
            antenv.axon_hooks = m
            sys.path.insert(0, "/root/.axon_site")
            from trn_agent_boot.trn_boot import _ntff_profile_via_ctypes

            m.set_axon_ntff_profile_hook(
                _ntff_profile_via_ctypes("/opt/axon/libaxon_pjrt.so")
            )
    except Exception as e:  # profiling is best-effort
        print(f"ntff hook install failed: {e}", file=sys.stderr)


def kernel(x, w1, b1, w2, b2, wf1, bf1, wf2, bf2, wf3, bf3):
    dr = CONV2_DR
    lo = CONV1_LO
    nc = _get_nc(dr, lo)
    _install_ntff_hook()
    from concourse import bass_utils

    # host-side relayout: xt[core][c*32+w, h*N+n] = x[core*N+n, c, h, w]
    xr = np.ascontiguousarray(
        x.reshape(NCORES, N, 3, 32, 32).transpose(0, 2, 4, 3, 1)
    ).reshape(NCORES, 96, 32 * N)

    t1 = _build_t1(w1).astype(np.float16)
    t2 = _build_t2(w2)  # [84, 800] f32, col = kh*160 + wop*80 + m
    f1 = _build_f1(wf1)  # [80, 600] f32, col = h5*120 + f
    f2 = np.ascontiguousarray(_binarize(wf2).T).astype(BF16)  # [120, 84]
    f3 = np.ascontiguousarray(_binarize(wf3).T).astype(BF16)  # [84, 10]
    b1v = np.tile(b1.astype(np.float32), 14).reshape(84, 1)  # part = wo2*6+o
    b2v = np.tile(b2.astype(np.float32), 5).reshape(80, 1)  # part = wo2*16+o
    shared = {
        "t1": t1, "f2": f2, "f3": f3,
        "b1v": b1v, "b2v": b2v,
        "bf1v": bf1.astype(np.float32).reshape(120, 1),
        "bf2v": bf2.astype(np.float32).reshape(84, 1),
        "bf3v": bf3.astype(np.float32).reshape(10, 1),
    }
    if dr:
        # t2a[p, pair*320 + wop*160 + i*80 + m] = t2[p, (2*pair+i)*160 + wop*80 + m]
        t2a = np.zeros((84, 640), np.float32)
        for pair in range(2):
            for wop in range(2):
                for i in range(2):
                    t2a[:, pair * 320 + wop * 160 + i * 80 : pair * 320 + wop * 160 + i * 80 + 80] = \
                        t2[:, (2 * pair + i) * 160 + wop * 80 : (2 * pair + i) * 160 + wop * 80 + 80]
        shared["t2a"] = t2a.astype(F8E4)
        shared["t2b"] = np.ascontiguousarray(t2[:, 640:800]).astype(F8E4)
        # f1d[p, pair*256 + i*128 + f] = f1[p, (2*pair+i)*120 + f] (f<120; pad 8 zeros)
        f1d = np.zeros((80, 512), np.float32)
        for pair in range(2):
            for i in range(2):
                f1d[:, pair * 256 + i * 128 : pair * 256 + i * 128 + 120] = \
                    f1[:, (2 * pair + i) * 120 : (2 * pair + i) * 120 + 120]
        shared["f1d"] = f1d.astype(F8E4)
        f14 = np.zeros((80, 128), np.float32)
        f14[:, 0:120] = f1[:, 480:600]
        shared["f14"] = f14.astype(F8E4)
    else:
        shared["t2"] = t2.astype(BF16)
        shared["f1"] = f1.astype(BF16)

    xh = xr.astype(np.float16)
    if lo == "dr8":
        s = np.float32(2.0 ** LOSC)
        si = np.float32(2.0 ** -LOSC)
        r1 = xr - xh.astype(np.float32)
        l1 = (r1 * s).astype(F8E4)
        r2 = r1 - l1.astype(np.float32) * si
        l2 = (r2 * s).astype(F8E4)
        # xlo per 2-row tile k: [l1 rows 2k,2k+1 | l2 rows 2k,2k+1]
        l1t = l1.reshape(NCORES, 96, 16, 2 * N)
        l2t = l2.reshape(NCORES, 96, 16, 2 * N)
        xlo = np.stack([l1t, l2t], axis=3).reshape(NCORES, 96, 64 * N)
        # t1lo[p, (kh*2+par)*192 + i*96 + m] = t1[p, kh*168+par*84+m] * 2^-LOSC
        t1f = _build_t1(w1)
        t1lo = np.zeros((96, 1920), np.float32)
        for kh in range(5):
            for par in range(2):
                src = t1f[:, kh * 168 + par * 84 : kh * 168 + par * 84 + 84] * si
                for i in range(2):
                    t1lo[:, (kh * 2 + par) * 192 + i * 96 : (kh * 2 + par) * 192 + i * 96 + 84] = src
        shared["t1lo"] = t1lo.astype(ml_dtypes.float8_e5m2)
        in_maps = [
            dict(shared, xh=np.ascontiguousarray(xh[i]), xlo=np.ascontiguousarray(xlo[i]))
            for i in range(NCORES)
        ]
    else:
        xl = (xr - xh.astype(np.float32)).astype(np.float16)
        in_maps = [
            dict(shared, xh=np.ascontiguousarray(xh[i]), xl=np.ascontiguousarray(xl[i]))
            for i in range(NCORES)
        ]

    res = bass_utils.run_bass_kernel_spmd(
        nc, in_maps, core_ids=list(range(NCORES)),
        trace=bool(int(os.environ.get("KERNEL_TRACE", "0"))),
    )
    if res.exec_time_ns is not None:
        print(f"HW exec time: {res.exec_time_ns} ns")
    out = np.stack([r["out"] for r in res.results])  # [8, 10, N]
    return np.ascontiguousarray(out.transpose(0, 2, 1)).reshape(B, 10).astype(np.float32)


# revision 27
# speedup vs baseline: 1.1909x; 1.1909x over previous
"""BinaryLeNet5 forward on 8 TRN2 NeuronCores, pure data parallel (1024 imgs/core).

Mapping summary (per core):
  conv1: kh-accumulated banded-Toeplitz matmuls. lhsT T1[kh] = [96=(c,wi), 168=(mc,wop,wo2,o)]
         built host-side from binarized w1; rhs = direct row-slices of transposed input
         xt[(c,w), (h,n)]. conv1 touches real-valued x, and the net is chaotic in the
         conv1 signs (even 2^-17 rounding flips enough signs to fail the 2e-2 gate), so
         the rhs is an fp16 hi+lo split (residual 2^-22 -> measured 0 final error),
         two accumulating passes per kh at 1 cycle/row each (fp32 native is 4 cyc/row).
  pool+sign: maxpool ho-pairs are the two contiguous halves of each PSUM tile and
         wo-pairs are the two wo-parity PSUM tiles -> 3 contiguous DVE tensor_max ops
         (strided reduce_max was ~1.8x slower), then ACT Sign with f32 per-partition
         bias (bias exact; commutes with max).
  conv2: inputs/weights exactly +-1 -> fp8e4 exact; kh pairs {0,1},{2,3} run as
         DoubleRow matmuls (2 fp8 weights/PE cell, K_eff=168, ~2x column rate), kh4 as
         a plain fp8 matmul. f32 PSUM accumulation of +-1 sums is exact.
  fc1/fc2/fc3: +-1 bf16 matmuls, f32 PSUM, biases applied in f32 via ACT bias operand.
  hardtanh drops out everywhere: sign(clip(x)) == sign(x), max(clip) == clip(max).

DMA: two HW DGE issue queues (sync=hi tiles, scalar=lo tiles+weights); input split
into 2-h-row tiles so the first matmul waits on ~0.55MB, not the full 12.6MB.

Output written as [10, 1024] per core, transposed/stacked on host.
"""

import os
import sys

import numpy as np

sys.path.insert(0, "/opt/trn_rl_repo")

import ml_dtypes  # noqa: E402

BF16 = ml_dtypes.bfloat16
F8E4 = ml_dtypes.float8_e4m3

B = 8192
NCORES = 8
N = B // NCORES  # 1024 images per core
NBLK = 2  # n blocks of 512 columns
NB = N // NBLK  # 512

CONV2_DR = int(os.environ.get("CONV2_DR", "1"))  # fp8 DoubleRow conv2
# conv1 lo-pass mode: "dr8" = both fp8 lo-terms in one DoubleRow matmul
# (descale 2^-14 folded into e5m2 weights; rel err ~0.011, deterministic),
# "f16" = single fp16 lo pass (exact).
CONV1_LO = os.environ.get("CONV1_LO", "dr8")
LOSC = 14  # lo-term scale exponent


def _binarize(w):
    return np.where(w >= 0, 1.0, -1.0).astype(np.float32)


def _build_t1(w1):
    # t1[c*32+wi, kh*168 + par*84 + wo2*6 + o] = w1b[o,c,kh,kw]
    #   wo = 2*wo2 + par (par = wo parity), kw = wi - wo, valid 0<=kw<5
    # Chunk A (par=0) = all even wo, chunk B = all odd wo -> the 2x2 maxpool's
    # wo-pair max is a same-partition tensor_max of the two chunk results.
    w1b = _binarize(w1)  # [6,3,5,5]
    t1 = np.zeros((96, 5 * 168), np.float32)
    for kh in range(5):
        for par in range(2):
            for wo2 in range(14):
                wo = 2 * wo2 + par
                for o in range(6):
                    col = kh * 168 + par * 84 + wo2 * 6 + o
                    for c in range(3):
                        for kw in range(5):
                            wi = wo + kw
                            if wi < 32:
                                t1[c * 32 + wi, col] = w1b[o, c, kh, kw]
    return t1


def _build_t2(w2):
    # t2[w2*6+c, kh*160 + wop*80 + wo2*16 + o] = w2b[o,c,kh,kw], kw = w2-(2*wo2+wop)
    w2b = _binarize(w2)  # [16,6,5,5]
    t2 = np.zeros((84, 5 * 160), np.float32)
    for kh in range(5):
        for wop in range(2):
            for wo2 in range(5):
                wo = 2 * wo2 + wop
                for o in range(16):
                    col = kh * 160 + wop * 80 + wo2 * 16 + o
                    for c in range(6):
                        for kw in range(5):
                            w2i = wo + kw
                            if w2i < 14:
                                t2[w2i * 6 + c, col] = w2b[o, c, kh, kw]
    return t2


def _build_f1(wf1):
    # f1[w*16+o, h*120+f] = wf1b[f, o*25+h*5+w]
    wf1b = _binarize(wf1)  # [120, 400]
    f1 = np.zeros((80, 5 * 120), np.float32)
    for h in range(5):
        for w in range(5):
            for o in range(16):
                f1[w * 16 + o, h * 120 : (h + 1) * 120] = wf1b[:, o * 25 + h * 5 + w]
    return f1


_CACHE = {}


def _get_nc(dr, lo):
    key = f"nc_dr{dr}_{lo}"
    if key in _CACHE:
        return _CACHE[key]
    import concourse.bacc as bacc
    import concourse.mybir as mybir
    import concourse.tile as tile

    f32 = mybir.dt.float32
    f16 = mybir.dt.float16
    bf16 = mybir.dt.bfloat16
    f8e4 = mybir.dt.float8e4
    f8e5 = mybir.dt.float8e5
    x2dt = f8e4 if dr else bf16

    nc = bacc.Bacc()
    xh_d = nc.dram_tensor("xh", [96, 32 * N], f16, kind="ExternalInput")
    if lo == "dr8":
        # per 2-row tile k: [l1 row0 | l1 row1 | l2 row0 | l2 row1], N cols each
        xlo_d = nc.dram_tensor("xlo", [96, 64 * N], f8e4, kind="ExternalInput")
        # per (kh,par): [i0: 96 cols | i1: 96 cols], both = t1 slice * 2^-LOSC,
        # M padded 84->96 (DR weight step must be %16==0)
        t1lo_d = nc.dram_tensor("t1lo", [96, 1920], f8e5, kind="ExternalInput")
    else:
        xl_d = nc.dram_tensor("xl", [96, 32 * N], f16, kind="ExternalInput")
    t1_d = nc.dram_tensor("t1", [96, 840], f16, kind="ExternalInput")
    if dr:
        t2a_d = nc.dram_tensor("t2a", [84, 640], f8e4, kind="ExternalInput")
        t2b_d = nc.dram_tensor("t2b", [84, 160], f8e4, kind="ExternalInput")
        f1d_d = nc.dram_tensor("f1d", [80, 512], f8e4, kind="ExternalInput")
        f14_d = nc.dram_tensor("f14", [80, 128], f8e4, kind="ExternalInput")
    else:
        t2_d = nc.dram_tensor("t2", [84, 800], bf16, kind="ExternalInput")
        f1_d = nc.dram_tensor("f1", [80, 600], bf16, kind="ExternalInput")
    f2_d = nc.dram_tensor("f2", [120, 84], bf16, kind="ExternalInput")
    f3_d = nc.dram_tensor("f3", [84, 10], bf16, kind="ExternalInput")
    b1_d = nc.dram_tensor("b1v", [84, 1], f32, kind="ExternalInput")
    b2_d = nc.dram_tensor("b2v", [80, 1], f32, kind="ExternalInput")
    bf1_d = nc.dram_tensor("bf1v", [120, 1], f32, kind="ExternalInput")
    bf2_d = nc.dram_tensor("bf2v", [84, 1], f32, kind="ExternalInput")
    bf3_d = nc.dram_tensor("bf3v", [10, 1], f32, kind="ExternalInput")
    out_d = nc.dram_tensor("out", [10, N], f32, kind="ExternalOutput")

    HCH = 2  # h rows per xt sbuf tile
    NT = 32 // HCH  # 16 tiles per hi/lo
    with tile.TileContext(nc) as tc:
        with (
            tc.tile_pool(name="xtp", bufs=1) as xtp,
            tc.tile_pool(name="wts", bufs=1) as wts,
            tc.tile_pool(name="acts", bufs=1) as acts,
            tc.tile_pool(name="ev", bufs=3) as ev,
            tc.tile_pool(name="ps", bufs=4, space="PSUM") as ps,
        ):
            # ---- DMA plan: sync queue = t1 + hi tiles (conv1 critical path);
            # scalar queue = first lo tiles, then the conv2/fc weights+biases
            # (needed from ~30us on), then the remaining lo tiles. ----
            t1s = wts.tile([96, 840], f16, tag="t1")
            nc.scalar.dma_start(out=t1s, in_=t1_d[:, :])
            if lo == "dr8":
                t1los = wts.tile([96, 1920], f8e5, tag="t1lo")
                nc.scalar.dma_start(out=t1los, in_=t1lo_d[:, :])

            LCH = 2 * HCH * N if lo == "dr8" else HCH * N  # lo cols per tile
            lodt = f8e4 if lo == "dr8" else f16
            lod = xlo_d if lo == "dr8" else xl_d
            xhs, xls = [], []
            for k in range(NT):
                th = xtp.tile([96, HCH * N], f16, tag=f"xh{k}", name=f"xh{k}")
                nc.sync.dma_start(out=th, in_=xh_d[:, k * HCH * N : (k + 1) * HCH * N])
                xhs.append(th)
            for k in range(3):
                tl = xtp.tile([96, LCH], lodt, tag=f"xl{k}", name=f"xl{k}")
                nc.scalar.dma_start(out=tl, in_=lod[:, k * LCH : (k + 1) * LCH])
                xls.append(tl)

            if dr:
                t2as = wts.tile([84, 640], f8e4, tag="t2a")
                nc.scalar.dma_start(out=t2as, in_=t2a_d[:, :])
                t2bs = wts.tile([84, 160], f8e4, tag="t2b")
                nc.scalar.dma_start(out=t2bs, in_=t2b_d[:, :])
                f1ds = wts.tile([80, 512], f8e4, tag="f1d")
                nc.scalar.dma_start(out=f1ds, in_=f1d_d[:, :])
                f14s = wts.tile([80, 128], f8e4, tag="f14")
                nc.scalar.dma_start(out=f14s, in_=f14_d[:, :])
            else:
                t2s = wts.tile([84, 800], bf16, tag="t2")
                nc.scalar.dma_start(out=t2s, in_=t2_d[:, :])
                f1s = wts.tile([80, 600], bf16, tag="f1")
                nc.scalar.dma_start(out=f1s, in_=f1_d[:, :])
            f2s = wts.tile([120, 84], bf16, tag="f2")
            nc.scalar.dma_start(out=f2s, in_=f2_d[:, :])
            f3s = wts.tile([84, 10], bf16, tag="f3")
            nc.scalar.dma_start(out=f3s, in_=f3_d[:, :])
            b1s = wts.tile([84, 1], f32, tag="b1")
            nc.scalar.dma_start(out=b1s, in_=b1_d[:, :])
            b2s = wts.tile([80, 1], f32, tag="b2")
            nc.scalar.dma_start(out=b2s, in_=b2_d[:, :])
            bf1s = wts.tile([120, 1], f32, tag="bf1")
            nc.scalar.dma_start(out=bf1s, in_=bf1_d[:, :])
            bf2s = wts.tile([84, 1], f32, tag="bf2")
            nc.scalar.dma_start(out=bf2s, in_=bf2_d[:, :])
            bf3s = wts.tile([10, 1], f32, tag="bf3")
            nc.scalar.dma_start(out=bf3s, in_=bf3_d[:, :])

            for k in range(3, NT):
                tl = xtp.tile([96, LCH], lodt, tag=f"xl{k}", name=f"xl{k}")
                nc.scalar.dma_start(out=tl, in_=lod[:, k * LCH : (k + 1) * LCH])
                xls.append(tl)

            def xhi(h, nb):  # hi rhs [96, NB] for input row h
                off = (h % HCH) * N + nb * NB
                return xhs[h // HCH][:, off : off + NB]

            def xlo(h, nb):
                if lo == "dr8":  # [96, 2, NB]: (l1, l2) pair for DoubleRow
                    t = xls[h // HCH].rearrange("p (i c) -> p i c", i=2)
                    off = (h % HCH) * N + nb * NB
                    return t[:, :, off : off + NB]
                off = (h % HCH) * N + nb * NB
                return xls[h // HCH][:, off : off + NB]

            # One consumer-engine 'touch' per DMA'd bias tile: the touch op
            # carries the DMA wait, so later ops on that engine need no extra
            # wait slot (TRN2 engine instructions have a single wait slot).
            tb1 = wts.tile([84, 1], f32, tag="tb1")
            nc.scalar.copy(tb1, b1s)
            tb2 = wts.tile([80, 1], f32, tag="tb2")
            nc.scalar.copy(tb2, b2s)
            tb3 = wts.tile([120, 1], f32, tag="tb3")
            nc.scalar.copy(tb3, bf1s)
            tb4 = wts.tile([84, 1], f32, tag="tb4")
            nc.scalar.copy(tb4, bf2s)
            tb5 = wts.tile([10, 1], f32, tag="tb5")
            nc.vector.tensor_copy(tb5, bf3s)

            x2 = acts.tile([84, 14 * N], x2dt, tag="x2")
            x3 = acts.tile([80, 5 * N], f8e4 if dr else bf16, tag="x3")
            x4 = acts.tile([120, N], bf16, tag="x4")
            x5 = acts.tile([84, N], bf16, tag="x5")
            outs = acts.tile([10, N], f32, tag="outs")

            # ---- conv1 + pool + sign -> x2 ----
            # psum tile [84, 1024] = (hop 2) x (n 512) blocks; ho-pair pooled as
            # tensor_max of the tile's two contiguous halves; wo-pair = tensor_max
            # of the two parity tiles (same partitions). No strided DVE reads.
            PM = 96 if lo == "dr8" else 84  # psum partitions (DR pads M 84->96)
            for ho2 in range(14):
                for nb in range(NBLK):
                    p = [ps.tile([PM, 2 * NB], f32, tag="ps", name="p1") for _ in range(2)]
                    for par in range(2):
                        if lo == "dr8":
                            # Phase 1: all 10 DoubleRow lo matmuls (same PE mode,
                            # weights switch only per kh — keeps LDW pipelined;
                            # interleaving DR and normal MMs cost ~250 cyc/pair).
                            # The 96-row kh0 DR opens the accumulation region.
                            for kh in range(5):
                                lod_lhs = t1los[
                                    :, (kh * 2 + par) * 192 : (kh * 2 + par) * 192 + 192
                                ].rearrange("p (i m) -> p i m", i=2)
                                for hop in range(2):
                                    h = 2 * ho2 + hop + kh
                                    nc.tensor.matmul(
                                        p[par][:, hop * NB : hop * NB + NB],
                                        lod_lhs, xlo(h, nb),
                                        start=(kh == 0), stop=False,
                                        perf_mode=mybir.MatmulPerfMode.DoubleRow,
                                    )
                            # Phase 2: all 10 fp16 hi matmuls accumulate on top.
                            for kh in range(5):
                                lhs = t1s[:, kh * 168 + par * 84 : kh * 168 + par * 84 + 84]
                                for hop in range(2):
                                    h = 2 * ho2 + hop + kh
                                    nc.tensor.matmul(
                                        p[par][0:84, hop * NB : hop * NB + NB],
                                        lhs, xhi(h, nb),
                                        start=False, stop=(kh == 4),
                                        skip_group_check=True,
                                    )
                        else:
                            for kh in range(5):
                                lhs = t1s[:, kh * 168 + par * 84 : kh * 168 + par * 84 + 84]
                                for hop in range(2):
                                    h = 2 * ho2 + hop + kh
                                    out84 = p[par][0:84, hop * NB : hop * NB + NB]
                                    nc.tensor.matmul(
                                        out84, lhs, xhi(h, nb),
                                        start=(kh == 0), stop=False,
                                    )
                                    nc.tensor.matmul(
                                        out84, lhs, xlo(h, nb),
                                        start=False, stop=(kh == 4),
                                    )
                    e1 = []
                    for par in range(2):
                        e = ev.tile([96, NB], f32, tag="ea", name="e1")[0:84]
                        nc.vector.reduce_max(e, p[par][0:84, :].rearrange("q (h n) -> q n h", h=2), axis=mybir.AxisListType.X)
                        e1.append(e)
                    e2 = ev.tile([96, NB], f32, tag="ec", name="e2")[0:84]
                    nc.vector.tensor_max(e2, e1[0], e1[1])
                    nc.scalar.sign(
                        x2[:, ho2 * N + nb * NB : ho2 * N + nb * NB + NB],
                        e2, bias=b1s,
                    )
            # ---- conv2 + pool + sign -> x3 ----
            x2r = x2.rearrange("p (h n) -> p h n", h=14)
            for ho2 in range(5):
                for nb in range(NBLK):
                    p2 = [ps.tile([80, 2 * NB], f32, tag="ps", name="p2") for _ in range(2)]
                    for wop in range(2):
                        if dr:
                            for pair in range(2):  # kh {0,1} and {2,3}
                                lhs = t2as[
                                    :, pair * 320 + wop * 160 : pair * 320 + wop * 160 + 160
                                ].rearrange("p (i m) -> p i m", i=2)
                                for hop in range(2):
                                    h0 = 2 * ho2 + hop + 2 * pair
                                    nc.tensor.matmul(
                                        p2[wop][:, hop * NB : hop * NB + NB], lhs,
                                        x2r[:, h0 : h0 + 2, nb * NB : nb * NB + NB],
                                        start=(pair == 0), stop=False,
                                        perf_mode=mybir.MatmulPerfMode.DoubleRow,
                                    )
                            lhs4 = t2bs[:, wop * 80 : wop * 80 + 80]
                            for hop in range(2):
                                h_in = 2 * ho2 + hop + 4
                                nc.tensor.matmul(
                                    p2[wop][:, hop * NB : hop * NB + NB], lhs4,
                                    x2[:, h_in * N + nb * NB : h_in * N + nb * NB + NB],
                                    start=False, stop=True,
                                )
                        else:
                            for kh in range(5):
                                lhs = t2s[:, kh * 160 + wop * 80 : kh * 160 + wop * 80 + 80]
                                for hop in range(2):
                                    h_in = 2 * ho2 + hop + kh
                                    nc.tensor.matmul(
                                        p2[wop][:, hop * NB : hop * NB + NB], lhs,
                                        x2[:, h_in * N + nb * NB : h_in * N + nb * NB + NB],
                                        start=(kh == 0), stop=(kh == 4),
                                    )
                    ew = []
                    for wop in range(2):
                        # conv2 PSUM values are exact small integers -> bf16 pooled
                        # tiles are exact and 16-bit DVE ops run at 2x rate.
                        e = ev.tile([96, NB], bf16, tag="eb", name="e3")[0:80]
                        nc.vector.reduce_max(e, p2[wop].rearrange("q (h n) -> q n h", h=2), axis=mybir.AxisListType.X)
                        ew.append(e)
                    e4 = ev.tile([96, NB], bf16, tag="ed", name="e4")[0:80]
                    nc.vector.tensor_max(e4, ew[0], ew[1])
                    nc.scalar.sign(
                        x3[:, ho2 * N + nb * NB : ho2 * N + nb * NB + NB], e4, bias=b2s
                    )

            # ---- fc1 -> fc2 -> fc3 ----
            # 4 n-sub-blocks of 256 with stage-interleaving: shortens the exposed
            # serial MM->sign->MM chain at the kernel tail; fc1 h5-pairs {0,1},{2,3}
            # run as fp8 DoubleRow (M padded 120->128), h5=4 as a plain fp8 matmul.
            NBF = 2
            NF = N // NBF  # 512
            x3r = x3.rearrange("p (h n) -> p h n", h=5)
            p3 = [
                ps.tile([128 if dr else 120, NF], f32, tag="ps", name="p3")
                for _ in range(NBF)
            ]
            for nb in range(NBF):
                if dr:
                    for pair in range(2):
                        lhs = f1ds[:, pair * 256 : pair * 256 + 256].rearrange(
                            "p (i m) -> p i m", i=2
                        )
                        nc.tensor.matmul(
                            p3[nb], lhs,
                            x3r[:, 2 * pair : 2 * pair + 2, nb * NF : nb * NF + NF],
                            start=(pair == 0), stop=False,
                            perf_mode=mybir.MatmulPerfMode.DoubleRow,
                        )
                    nc.tensor.matmul(
                        p3[nb], f14s,
                        x3[:, 4 * N + nb * NF : 4 * N + nb * NF + NF],
                        start=False, stop=True,
                    )
                else:
                    for h5 in range(5):
                        nc.tensor.matmul(
                            p3[nb], f1s[:, h5 * 120 : (h5 + 1) * 120],
                            x3[:, h5 * N + nb * NF : h5 * N + nb * NF + NF],
                            start=(h5 == 0), stop=(h5 == 4),
                        )
            for nb in range(NBF):
                nc.scalar.sign(x4[:, nb * NF : nb * NF + NF], p3[nb][0:120], bias=bf1s)
            p4 = [ps.tile([84, NF], f32, tag="ps", name="p4") for _ in range(NBF)]
            for nb in range(NBF):
                nc.tensor.matmul(p4[nb], f2s, x4[:, nb * NF : nb * NF + NF], start=True, stop=True)
            for nb in range(NBF):
                nc.scalar.sign(x5[:, nb * NF : nb * NF + NF], p4[nb], bias=bf2s)
            p5 = [ps.tile([10, NF], f32, tag="ps", name="p5") for _ in range(NBF)]
            for nb in range(NBF):
                nc.tensor.matmul(p5[nb], f3s, x5[:, nb * NF : nb * NF + NF], start=True, stop=True)
            for nb in range(NBF):
                nc.vector.tensor_scalar_add(outs[:, nb * NF : nb * NF + NF], p5[nb], bf3s)
                nc.sync.dma_start(
                    out=out_d[:, nb * NF : nb * NF + NF],
                    in_=outs[:, nb * NF : nb * NF + NF],
                )

    nc.finalize()
    _CACHE[key] = nc
    return nc


def _install_ntff_hook():
    """The container's antenv stub lacks axon_hooks; synthesize it and register
    the ctypes-based NTFF profile hook from the axon boot module."""
    if "hook" in _CACHE:
        return
    _CACHE["hook"] = True
    try:
        import types
        import antenv

        if not hasattr(antenv, "axon_hooks"):
            store = {"h": None}
            m = types.ModuleType("antenv.axon_hooks")
            m.set_axon_ntff_profile_hook = lambda h: store.update(h=h)
            m.get_axon_ntff_profile_hook = lambda: store["h"]
            sys.modules["antenv.axon_hooks"] = m
            antenv.axon_hooks = m
            sys.path.insert(0, "/root/.axon_site")
            from trn_agent_boot.trn_boot import _ntff_profile_via_ctypes

            m.set_axon_ntff_profile_hook(
                _ntff_profile_via_ctypes("/opt/axon/libaxon_pjrt.so")
            )
    except Exception as e:  # profiling is best-effort
        print(f"ntff hook install failed: {e}", file=sys.stderr)


def kernel(x, w1, b1, w2, b2, wf1, bf1, wf2, bf2, wf3, bf3):
    dr = CONV2_DR
    lo = CONV1_LO
    nc = _get_nc(dr, lo)
    _install_ntff_hook()
    from concourse import bass_utils

    # host-side relayout: xt[core][c*32+w, h*N+n] = x[core*N+n, c, h, w]
    xr = np.ascontiguousarray(
        x.reshape(NCORES, N, 3, 32, 32).transpose(0, 2, 4, 3, 1)
    ).reshape(NCORES, 96, 32 * N)

    t1 = _build_t1(w1).astype(np.float16)
    t2 = _build_t2(w2)  # [84, 800] f32, col = kh*160 + wop*80 + m
    f1 = _build_f1(wf1)  # [80, 600] f32, col = h5*120 + f
    f2 = np.ascontiguousarray(_binarize(wf2).T).astype(BF16)  # [120, 84]
    f3 = np.ascontiguousarray(_binarize(wf3).T).astype(BF16)  # [84, 10]
    b1v = np.tile(b1.astype(np.float32), 14).reshape(84, 1)  # part = wo2*6+o
    b2v = np.tile(b2.astype(np.float32), 5).reshape(80, 1)  # part = wo2*16+o
    shared = {
        "t1": t1, "f2": f2, "f3": f3,
        "b1v": b1v, "b2v": b2v,
        "bf1v": bf1.astype(np.float32).reshape(120, 1),
        "bf2v": bf2.astype(np.float32).reshape(84, 1),
        "bf3v": bf3.astype(np.float32).reshape(10, 1),
    }
    if dr:
        # t2a[p, pair*320 + wop*160 + i*80 + m] = t2[p, (2*pair+i)*160 + wop*80 + m]
        t2a = np.zeros((84, 640), np.float32)
        for pair in range(2):
            for wop in range(2):
                for i in range(2):
                    t2a[:, pair * 320 + wop * 160 + i * 80 : pair * 320 + wop * 160 + i * 80 + 80] = \
                        t2[:, (2 * pair + i) * 160 + wop * 80 : (2 * pair + i) * 160 + wop * 80 + 80]
        shared["t2a"] = t2a.astype(F8E4)
        shared["t2b"] = np.ascontiguousarray(t2[:, 640:800]).astype(F8E4)
        # f1d[p, pair*256 + i*128 + f] = f1[p, (2*pair+i)*120 + f] (f<120; pad 8 zeros)
        f1d = np.zeros((80, 512), np.float32)
        for pair in range(2):
            for i in range(2):
                f1d[:, pair * 256 + i * 128 : pair * 256 + i * 128 + 120] = \
                    f1[:, (2 * pair + i) * 120 : (2 * pair + i) * 120 + 120]
        shared["f1d"] = f1d.astype(F8E4)
        f14 = np.zeros((80, 128), np.float32)
        f14[:, 0:120] = f1[:, 480:600]
        shared["f14"] = f14.astype(F8E4)
    else:
        shared["t2"] = t2.astype(BF16)
        shared["f1"] = f1.astype(BF16)

    xh = xr.astype(np.float16)
    if lo == "dr8":
        s = np.float32(2.0 ** LOSC)
        si = np.float32(2.0 ** -LOSC)
        r1 = xr - xh.astype(np.float32)
        l1 = (r1 * s).astype(F8E4)
        r2 = r1 - l1.astype(np.float32) * si
        l2 = (r2 * s).astype(F8E4)
        # xlo per 2-row tile k: [l1 rows 2k,2k+1 | l2 rows 2k,2k+1]
        l1t = l1.reshape(NCORES, 96, 16, 2 * N)
        l2t = l2.reshape(NCORES, 96, 16, 2 * N)
        xlo = np.stack([l1t, l2t], axis=3).reshape(NCORES, 96, 64 * N)
        # t1lo[p, (kh*2+par)*192 + i*96 + m] = t1[p, kh*168+par*84+m] * 2^-LOSC
        t1f = _build_t1(w1)
        t1lo = np.zeros((96, 1920), np.float32)
        for kh in range(5):
            for par in range(2):
                src = t1f[:, kh * 168 + par * 84 : kh * 168 + par * 84 + 84] * si
                for i in range(2):
                    t1lo[:, (kh * 2 + par) * 192 + i * 96 : (kh * 2 + par) * 192 + i * 96 + 84] = src
        shared["t1lo"] = t1lo.astype(ml_dtypes.float8_e5m2)
        in_maps = [
            dict(shared, xh=np.ascontiguousarray(xh[i]), xlo=np.ascontiguousarray(xlo[i]))
            for i in range(NCORES)
        ]
    else:
        xl = (xr - xh.astype(np.float32)).astype(np.float16)
        in_maps = [
            dict(shared, xh=np.ascontiguousarray(xh[i]), xl=np.ascontiguousarray(xl[i]))
            for i in range(NCORES)
        ]

    res = bass_utils.run_bass_kernel_spmd(
        nc, in_maps, core_ids=list(range(NCORES)),
        trace=bool(int(os.environ.get("KERNEL_TRACE", "0"))),
    )
    if res.exec_time_ns is not None:
        print(f"HW exec time: {res.exec_time_ns} ns")
    out = np.stack([r["out"] for r in res.results])  # [8, 10, N]
    return np.ascontiguousarray(out.transpose(0, 2, 1)).reshape(B, 10).astype(np.float32)


# revision 28
# speedup vs baseline: 1.2044x; 1.0113x over previous
"""BinaryLeNet5 forward on 8 TRN2 NeuronCores, pure data parallel (1024 imgs/core).

Mapping summary (per core):
  conv1: kh-accumulated banded-Toeplitz matmuls. lhsT T1[kh] = [96=(c,wi), 168=(mc,wop,wo2,o)]
         built host-side from binarized w1; rhs = direct row-slices of transposed input
         xt[(c,w), (h,n)]. conv1 touches real-valued x, and the net is chaotic in the
         conv1 signs (even 2^-17 rounding flips enough signs to fail the 2e-2 gate), so
         the rhs is an fp16 hi+lo split (residual 2^-22 -> measured 0 final error),
         two accumulating passes per kh at 1 cycle/row each (fp32 native is 4 cyc/row).
  pool+sign: maxpool ho-pairs are the two contiguous halves of each PSUM tile and
         wo-pairs are the two wo-parity PSUM tiles -> 3 contiguous DVE tensor_max ops
         (strided reduce_max was ~1.8x slower), then ACT Sign with f32 per-partition
         bias (bias exact; commutes with max).
  conv2: inputs/weights exactly +-1 -> fp8e4 exact; kh pairs {0,1},{2,3} run as
         DoubleRow matmuls (2 fp8 weights/PE cell, K_eff=168, ~2x column rate), kh4 as
         a plain fp8 matmul. f32 PSUM accumulation of +-1 sums is exact.
  fc1/fc2/fc3: +-1 bf16 matmuls, f32 PSUM, biases applied in f32 via ACT bias operand.
  hardtanh drops out everywhere: sign(clip(x)) == sign(x), max(clip) == clip(max).

DMA: two HW DGE issue queues (sync=hi tiles, scalar=lo tiles+weights); input split
into 2-h-row tiles so the first matmul waits on ~0.55MB, not the full 12.6MB.

Output written as [10, 1024] per core, transposed/stacked on host.
"""

import os
import sys

import numpy as np

sys.path.insert(0, "/opt/trn_rl_repo")

import ml_dtypes  # noqa: E402

BF16 = ml_dtypes.bfloat16
F8E4 = ml_dtypes.float8_e4m3

B = 8192
NCORES = 8
N = B // NCORES  # 1024 images per core
NBLK = 2  # n blocks of 512 columns
NB = N // NBLK  # 512

CONV2_DR = int(os.environ.get("CONV2_DR", "1"))  # fp8 DoubleRow conv2
# conv1 lo-pass mode: "f16" (default) = single fp16 lo pass (exact). "dr8" =
# two fp8 lo-terms in one DoubleRow matmul (descale 2^-14 via e5m2 weights) —
# measured SAME speed as f16 (DR packs K, it does not double the column rate)
# with worse (though passing) accuracy: rel err 0.0111. Kept for reference.
CONV1_LO = os.environ.get("CONV1_LO", "f16")
LOSC = 14  # lo-term scale exponent


def _binarize(w):
    return np.where(w >= 0, 1.0, -1.0).astype(np.float32)


def _build_t1(w1):
    # t1[c*32+wi, kh*168 + par*84 + wo2*6 + o] = w1b[o,c,kh,kw]
    #   wo = 2*wo2 + par (par = wo parity), kw = wi - wo, valid 0<=kw<5
    # Chunk A (par=0) = all even wo, chunk B = all odd wo -> the 2x2 maxpool's
    # wo-pair max is a same-partition tensor_max of the two chunk results.
    w1b = _binarize(w1)  # [6,3,5,5]
    t1 = np.zeros((96, 5 * 168), np.float32)
    for kh in range(5):
        for par in range(2):
            for wo2 in range(14):
                wo = 2 * wo2 + par
                for o in range(6):
                    col = kh * 168 + par * 84 + wo2 * 6 + o
                    for c in range(3):
                        for kw in range(5):
                            wi = wo + kw
                            if wi < 32:
                                t1[c * 32 + wi, col] = w1b[o, c, kh, kw]
    return t1


def _build_t2(w2):
    # t2[w2*6+c, kh*160 + wop*80 + wo2*16 + o] = w2b[o,c,kh,kw], kw = w2-(2*wo2+wop)
    w2b = _binarize(w2)  # [16,6,5,5]
    t2 = np.zeros((84, 5 * 160), np.float32)
    for kh in range(5):
        for wop in range(2):
            for wo2 in range(5):
                wo = 2 * wo2 + wop
                for o in range(16):
                    col = kh * 160 + wop * 80 + wo2 * 16 + o
                    for c in range(6):
                        for kw in range(5):
                            w2i = wo + kw
                            if w2i < 14:
                                t2[w2i * 6 + c, col] = w2b[o, c, kh, kw]
    return t2


def _build_f1(wf1):
    # f1[w*16+o, h*120+f] = wf1b[f, o*25+h*5+w]
    wf1b = _binarize(wf1)  # [120, 400]
    f1 = np.zeros((80, 5 * 120), np.float32)
    for h in range(5):
        for w in range(5):
            for o in range(16):
                f1[w * 16 + o, h * 120 : (h + 1) * 120] = wf1b[:, o * 25 + h * 5 + w]
    return f1


_CACHE = {}


def _get_nc(dr, lo):
    key = f"nc_dr{dr}_{lo}"
    if key in _CACHE:
        return _CACHE[key]
    import concourse.bacc as bacc
    import concourse.mybir as mybir
    import concourse.tile as tile

    f32 = mybir.dt.float32
    f16 = mybir.dt.float16
    bf16 = mybir.dt.bfloat16
    f8e4 = mybir.dt.float8e4
    f8e5 = mybir.dt.float8e5
    x2dt = f8e4 if dr else bf16

    nc = bacc.Bacc()
    xh_d = nc.dram_tensor("xh", [96, 32 * N], f16, kind="ExternalInput")
    if lo == "dr8":
        # per 2-row tile k: [l1 row0 | l1 row1 | l2 row0 | l2 row1], N cols each
        xlo_d = nc.dram_tensor("xlo", [96, 64 * N], f8e4, kind="ExternalInput")
        # per (kh,par): [i0: 96 cols | i1: 96 cols], both = t1 slice * 2^-LOSC,
        # M padded 84->96 (DR weight step must be %16==0)
        t1lo_d = nc.dram_tensor("t1lo", [96, 1920], f8e5, kind="ExternalInput")
    else:
        xl_d = nc.dram_tensor("xl", [96, 32 * N], f16, kind="ExternalInput")
    t1_d = nc.dram_tensor("t1", [96, 840], f16, kind="ExternalInput")
    if dr:
        t2a_d = nc.dram_tensor("t2a", [84, 640], f8e4, kind="ExternalInput")
        t2b_d = nc.dram_tensor("t2b", [84, 160], f8e4, kind="ExternalInput")
        f1d_d = nc.dram_tensor("f1d", [80, 512], f8e4, kind="ExternalInput")
        f14_d = nc.dram_tensor("f14", [80, 128], f8e4, kind="ExternalInput")
    else:
        t2_d = nc.dram_tensor("t2", [84, 800], bf16, kind="ExternalInput")
        f1_d = nc.dram_tensor("f1", [80, 600], bf16, kind="ExternalInput")
    f2_d = nc.dram_tensor("f2", [120, 84], bf16, kind="ExternalInput")
    f3_d = nc.dram_tensor("f3", [84, 10], bf16, kind="ExternalInput")
    b1_d = nc.dram_tensor("b1v", [84, 1], f32, kind="ExternalInput")
    b2_d = nc.dram_tensor("b2v", [80, 1], f32, kind="ExternalInput")
    bf1_d = nc.dram_tensor("bf1v", [120, 1], f32, kind="ExternalInput")
    bf2_d = nc.dram_tensor("bf2v", [84, 1], f32, kind="ExternalInput")
    bf3_d = nc.dram_tensor("bf3v", [10, 1], f32, kind="ExternalInput")
    out_d = nc.dram_tensor("out", [10, N], f32, kind="ExternalOutput")

    HCH = 2  # h rows per xt sbuf tile
    NT = 32 // HCH  # 16 tiles per hi/lo
    with tile.TileContext(nc) as tc:
        with (
            tc.tile_pool(name="xtp", bufs=1) as xtp,
            tc.tile_pool(name="wts", bufs=1) as wts,
            tc.tile_pool(name="acts", bufs=1) as acts,
            tc.tile_pool(name="ev", bufs=3) as ev,
            tc.tile_pool(name="ps", bufs=4, space="PSUM") as ps,
        ):
            # ---- DMA plan: sync queue = t1 + hi tiles (conv1 critical path);
            # scalar queue = first lo tiles, then the conv2/fc weights+biases
            # (needed from ~30us on), then the remaining lo tiles. ----
            t1s = wts.tile([96, 840], f16, tag="t1")
            nc.scalar.dma_start(out=t1s, in_=t1_d[:, :])
            if lo == "dr8":
                t1los = wts.tile([96, 1920], f8e5, tag="t1lo")
                nc.scalar.dma_start(out=t1los, in_=t1lo_d[:, :])

            LCH = 2 * HCH * N if lo == "dr8" else HCH * N  # lo cols per tile
            lodt = f8e4 if lo == "dr8" else f16
            lod = xlo_d if lo == "dr8" else xl_d
            xhs, xls = [], []
            for k in range(NT):
                th = xtp.tile([96, HCH * N], f16, tag=f"xh{k}", name=f"xh{k}")
                nc.sync.dma_start(out=th, in_=xh_d[:, k * HCH * N : (k + 1) * HCH * N])
                xhs.append(th)
            for k in range(3):
                tl = xtp.tile([96, LCH], lodt, tag=f"xl{k}", name=f"xl{k}")
                nc.scalar.dma_start(out=tl, in_=lod[:, k * LCH : (k + 1) * LCH])
                xls.append(tl)

            if dr:
                t2as = wts.tile([84, 640], f8e4, tag="t2a")
                nc.scalar.dma_start(out=t2as, in_=t2a_d[:, :])
                t2bs = wts.tile([84, 160], f8e4, tag="t2b")
                nc.scalar.dma_start(out=t2bs, in_=t2b_d[:, :])
                f1ds = wts.tile([80, 512], f8e4, tag="f1d")
                nc.scalar.dma_start(out=f1ds, in_=f1d_d[:, :])
                f14s = wts.tile([80, 128], f8e4, tag="f14")
                nc.scalar.dma_start(out=f14s, in_=f14_d[:, :])
            else:
                t2s = wts.tile([84, 800], bf16, tag="t2")
                nc.scalar.dma_start(out=t2s, in_=t2_d[:, :])
                f1s = wts.tile([80, 600], bf16, tag="f1")
                nc.scalar.dma_start(out=f1s, in_=f1_d[:, :])
            f2s = wts.tile([120, 84], bf16, tag="f2")
            nc.scalar.dma_start(out=f2s, in_=f2_d[:, :])
            f3s = wts.tile([84, 10], bf16, tag="f3")
            nc.scalar.dma_start(out=f3s, in_=f3_d[:, :])
            b1s = wts.tile([84, 1], f32, tag="b1")
            nc.scalar.dma_start(out=b1s, in_=b1_d[:, :])
            b2s = wts.tile([80, 1], f32, tag="b2")
            nc.scalar.dma_start(out=b2s, in_=b2_d[:, :])
            bf1s = wts.tile([120, 1], f32, tag="bf1")
            nc.scalar.dma_start(out=bf1s, in_=bf1_d[:, :])
            bf2s = wts.tile([84, 1], f32, tag="bf2")
            nc.scalar.dma_start(out=bf2s, in_=bf2_d[:, :])
            bf3s = wts.tile([10, 1], f32, tag="bf3")
            nc.scalar.dma_start(out=bf3s, in_=bf3_d[:, :])

            for k in range(3, NT):
                tl = xtp.tile([96, LCH], lodt, tag=f"xl{k}", name=f"xl{k}")
                nc.scalar.dma_start(out=tl, in_=lod[:, k * LCH : (k + 1) * LCH])
                xls.append(tl)

            def xhi(h, nb):  # hi rhs [96, NB] for input row h
                off = (h % HCH) * N + nb * NB
                return xhs[h // HCH][:, off : off + NB]

            def xlo(h, nb):
                if lo == "dr8":  # [96, 2, NB]: (l1, l2) pair for DoubleRow
                    t = xls[h // HCH].rearrange("p (i c) -> p i c", i=2)
                    off = (h % HCH) * N + nb * NB
                    return t[:, :, off : off + NB]
                off = (h % HCH) * N + nb * NB
                return xls[h // HCH][:, off : off + NB]

            # One consumer-engine 'touch' per DMA'd bias tile: the touch op
            # carries the DMA wait, so later ops on that engine need no extra
            # wait slot (TRN2 engine instructions have a single wait slot).
            tb1 = wts.tile([84, 1], f32, tag="tb1")
            nc.scalar.copy(tb1, b1s)
            tb2 = wts.tile([80, 1], f32, tag="tb2")
            nc.scalar.copy(tb2, b2s)
            tb3 = wts.tile([120, 1], f32, tag="tb3")
            nc.scalar.copy(tb3, bf1s)
            tb4 = wts.tile([84, 1], f32, tag="tb4")
            nc.scalar.copy(tb4, bf2s)
            tb5 = wts.tile([10, 1], f32, tag="tb5")
            nc.vector.tensor_copy(tb5, bf3s)

            x2 = acts.tile([84, 14 * N], x2dt, tag="x2")
            x3 = acts.tile([80, 5 * N], f8e4 if dr else bf16, tag="x3")
            x4 = acts.tile([120, N], bf16, tag="x4")
            x5 = acts.tile([84, N], bf16, tag="x5")
            outs = acts.tile([10, N], f32, tag="outs")

            # ---- conv1 + pool + sign -> x2 ----
            # psum tile [84, 1024] = (hop 2) x (n 512) blocks; ho-pair pooled as
            # tensor_max of the tile's two contiguous halves; wo-pair = tensor_max
            # of the two parity tiles (same partitions). No strided DVE reads.
            PM = 96 if lo == "dr8" else 84  # psum partitions (DR pads M 84->96)
            for ho2 in range(14):
                for nb in range(NBLK):
                    p = [ps.tile([PM, 2 * NB], f32, tag="ps", name="p1") for _ in range(2)]
                    for par in range(2):
                        if lo == "dr8":
                            # Phase 1: all 10 DoubleRow lo matmuls (same PE mode,
                            # weights switch only per kh — keeps LDW pipelined;
                            # interleaving DR and normal MMs cost ~250 cyc/pair).
                            # The 96-row kh0 DR opens the accumulation region.
                            for kh in range(5):
                                lod_lhs = t1los[
                                    :, (kh * 2 + par) * 192 : (kh * 2 + par) * 192 + 192
                                ].rearrange("p (i m) -> p i m", i=2)
                                for hop in range(2):
                                    h = 2 * ho2 + hop + kh
                                    nc.tensor.matmul(
                                        p[par][:, hop * NB : hop * NB + NB],
                                        lod_lhs, xlo(h, nb),
                                        start=(kh == 0), stop=False,
                                        perf_mode=mybir.MatmulPerfMode.DoubleRow,
                                    )
                            # Phase 2: all 10 fp16 hi matmuls accumulate on top.
                            for kh in range(5):
                                lhs = t1s[:, kh * 168 + par * 84 : kh * 168 + par * 84 + 84]
                                for hop in range(2):
                                    h = 2 * ho2 + hop + kh
                                    nc.tensor.matmul(
                                        p[par][0:84, hop * NB : hop * NB + NB],
                                        lhs, xhi(h, nb),
                                        start=False, stop=(kh == 4),
                                        skip_group_check=True,
                                    )
                        else:
                            for kh in range(5):
                                lhs = t1s[:, kh * 168 + par * 84 : kh * 168 + par * 84 + 84]
                                for hop in range(2):
                                    h = 2 * ho2 + hop + kh
                                    out84 = p[par][0:84, hop * NB : hop * NB + NB]
                                    nc.tensor.matmul(
                                        out84, lhs, xhi(h, nb),
                                        start=(kh == 0), stop=False,
                                    )
                                    nc.tensor.matmul(
                                        out84, lhs, xlo(h, nb),
                                        start=False, stop=(kh == 4),
                                    )
                    e1 = []
                    for par in range(2):
                        e = ev.tile([96, NB], f32, tag="ea", name="e1")[0:84]
                        nc.vector.reduce_max(e, p[par][0:84, :].rearrange("q (h n) -> q n h", h=2), axis=mybir.AxisListType.X)
                        e1.append(e)
                    e2 = ev.tile([96, NB], f32, tag="ec", name="e2")[0:84]
                    nc.vector.tensor_max(e2, e1[0], e1[1])
                    nc.scalar.sign(
                        x2[:, ho2 * N + nb * NB : ho2 * N + nb * NB + NB],
                        e2, bias=b1s,
                    )
            # ---- conv2 + pool + sign -> x3 ----
            x2r = x2.rearrange("p (h n) -> p h n", h=14)
            for ho2 in range(5):
                for nb in range(NBLK):
                    p2 = [ps.tile([80, 2 * NB], f32, tag="ps", name="p2") for _ in range(2)]
                    for wop in range(2):
                        if dr:
                            for pair in range(2):  # kh {0,1} and {2,3}
                                lhs = t2as[
                                    :, pair * 320 + wop * 160 : pair * 320 + wop * 160 + 160
                                ].rearrange("p (i m) -> p i m", i=2)
                                for hop in range(2):
                                    h0 = 2 * ho2 + hop + 2 * pair
                                    nc.tensor.matmul(
                                        p2[wop][:, hop * NB : hop * NB + NB], lhs,
                                        x2r[:, h0 : h0 + 2, nb * NB : nb * NB + NB],
                                        start=(pair == 0), stop=False,
                                        perf_mode=mybir.MatmulPerfMode.DoubleRow,
                                    )
                            lhs4 = t2bs[:, wop * 80 : wop * 80 + 80]
                            for hop in range(2):
                                h_in = 2 * ho2 + hop + 4
                                nc.tensor.matmul(
                                    p2[wop][:, hop * NB : hop * NB + NB], lhs4,
                                    x2[:, h_in * N + nb * NB : h_in * N + nb * NB + NB],
                                    start=False, stop=True,
                                )
                        else:
                            for kh in range(5):
                                lhs = t2s[:, kh * 160 + wop * 80 : kh * 160 + wop * 80 + 80]
                                for hop in range(2):
                                    h_in = 2 * ho2 + hop + kh
                                    nc.tensor.matmul(
                                        p2[wop][:, hop * NB : hop * NB + NB], lhs,
                                        x2[:, h_in * N + nb * NB : h_in * N + nb * NB + NB],
                                        start=(kh == 0), stop=(kh == 4),
                                    )
                    ew = []
                    for wop in range(2):
                        # conv2 PSUM values are exact small integers -> bf16 pooled
                        # tiles are exact and 16-bit DVE ops run at 2x rate.
                        e = ev.tile([96, NB], bf16, tag="eb", name="e3")[0:80]
                        nc.vector.reduce_max(e, p2[wop].rearrange("q (h n) -> q n h", h=2), axis=mybir.AxisListType.X)
                        ew.append(e)
                    e4 = ev.tile([96, NB], bf16, tag="ed", name="e4")[0:80]
                    nc.vector.tensor_max(e4, ew[0], ew[1])
                    nc.scalar.sign(
                        x3[:, ho2 * N + nb * NB : ho2 * N + nb * NB + NB], e4, bias=b2s
                    )

            # ---- fc1 -> fc2 -> fc3 ----
            # 4 n-sub-blocks of 256 with stage-interleaving: shortens the exposed
            # serial MM->sign->MM chain at the kernel tail; fc1 h5-pairs {0,1},{2,3}
            # run as fp8 DoubleRow (M padded 120->128), h5=4 as a plain fp8 matmul.
            NBF = 2
            NF = N // NBF  # 512
            x3r = x3.rearrange("p (h n) -> p h n", h=5)
            p3 = [
                ps.tile([128 if dr else 120, NF], f32, tag="ps", name="p3")
                for _ in range(NBF)
            ]
            for nb in range(NBF):
                if dr:
                    for pair in range(2):
                        lhs = f1ds[:, pair * 256 : pair * 256 + 256].rearrange(
                            "p (i m) -> p i m", i=2
                        )
                        nc.tensor.matmul(
                            p3[nb], lhs,
                            x3r[:, 2 * pair : 2 * pair + 2, nb * NF : nb * NF + NF],
                            start=(pair == 0), stop=False,
                            perf_mode=mybir.MatmulPerfMode.DoubleRow,
                        )
                    nc.tensor.matmul(
                        p3[nb], f14s,
                        x3[:, 4 * N + nb * NF : 4 * N + nb * NF + NF],
                        start=False, stop=True,
                    )
                else:
                    for h5 in range(5):
                        nc.tensor.matmul(
                            p3[nb], f1s[:, h5 * 120 : (h5 + 1) * 120],
                            x3[:, h5 * N + nb * NF : h5 * N + nb * NF + NF],
                            start=(h5 == 0), stop=(h5 == 4),
                        )
            for nb in range(NBF):
                nc.scalar.sign(x4[:, nb * NF : nb * NF + NF], p3[nb][0:120], bias=bf1s)
            p4 = [ps.tile([84, NF], f32, tag="ps", name="p4") for _ in range(NBF)]
            for nb in range(NBF):
                nc.tensor.matmul(p4[nb], f2s, x4[:, nb * NF : nb * NF + NF], start=True, stop=True)
            for nb in range(NBF):
                nc.scalar.sign(x5[:, nb * NF : nb * NF + NF], p4[nb], bias=bf2s)
            p5 = [ps.tile([10, NF], f32, tag="ps", name="p5") for _ in range(NBF)]
            for nb in range(NBF):
                nc.tensor.matmul(p5[nb], f3s, x5[:, nb * NF : nb * NF + NF], start=True, stop=True)
            for nb in range(NBF):
                nc.vector.tensor_scalar_add(outs[:, nb * NF : nb * NF + NF], p5[nb], bf3s)
                nc.sync.dma_start(
                    out=out_d[:, nb * NF : nb * NF + NF],
                    in_=outs[:, nb * NF : nb * NF + NF],
                )

    nc.finalize()
    _CACHE[key] = nc
    return nc


def _install_ntff_hook():
    """The container's antenv stub lacks axon_hooks; synthesize it and register
    the ctypes-based NTFF profile hook from the axon boot module."""
    if "hook" in _CACHE:
        return
    _CACHE["hook"] = True
    try:
        import types
        import antenv

        if not hasattr(antenv, "axon_hooks"):
            store = {"h": None}
            m = types.ModuleType("antenv.axon_hooks")
            m.set_axon_ntff_profile_hook = lambda h: store.update(h=h)
            m.get_axon_ntff_profile_hook = lambda: store["h"]
            sys.modules["antenv.axon_hooks"] = m
            antenv.axon_hooks = m
            sys.path.insert(0, "/root/.axon_site")
            from trn_agent_boot.trn_boot import _ntff_profile_via_ctypes

            m.set_axon_ntff_profile_hook(
                _ntff_profile_via_ctypes("/opt/axon/libaxon_pjrt.so")
            )
    except Exception as e:  # profiling is best-effort
        print(f"ntff hook install failed: {e}", file=sys.stderr)


def kernel(x, w1, b1, w2, b2, wf1, bf1, wf2, bf2, wf3, bf3):
    dr = CONV2_DR
    lo = CONV1_LO
    nc = _get_nc(dr, lo)
    _install_ntff_hook()
    from concourse import bass_utils

    # host-side relayout: xt[core][c*32+w, h*N+n] = x[core*N+n, c, h, w]
    xr = np.ascontiguousarray(
        x.reshape(NCORES, N, 3, 32, 32).transpose(0, 2, 4, 3, 1)
    ).reshape(NCORES, 96, 32 * N)

    t1 = _build_t1(w1).astype(np.float16)
    t2 = _build_t2(w2)  # [84, 800] f32, col = kh*160 + wop*80 + m
    f1 = _build_f1(wf1)  # [80, 600] f32, col = h5*120 + f
    f2 = np.ascontiguousarray(_binarize(wf2).T).astype(BF16)  # [120, 84]
    f3 = np.ascontiguousarray(_binarize(wf3).T).astype(BF16)  # [84, 10]
    b1v = np.tile(b1.astype(np.float32), 14).reshape(84, 1)  # part = wo2*6+o
    b2v = np.tile(b2.astype(np.float32), 5).reshape(80, 1)  # part = wo2*16+o
    shared = {
        "t1": t1, "f2": f2, "f3": f3,
        "b1v": b1v, "b2v": b2v,
        "bf1v": bf1.astype(np.float32).reshape(120, 1),
        "bf2v": bf2.astype(np.float32).reshape(84, 1),
        "bf3v": bf3.astype(np.float32).reshape(10, 1),
    }
    if dr:
        # t2a[p, pair*320 + wop*160 + i*80 + m] = t2[p, (2*pair+i)*160 + wop*80 + m]
        t2a = np.zeros((84, 640), np.float32)
        for pair in range(2):
            for wop in range(2):
                for i in range(2):
                    t2a[:, pair * 320 + wop * 160 + i * 80 : pair * 320 + wop * 160 + i * 80 + 80] = \
                        t2[:, (2 * pair + i) * 160 + wop * 80 : (2 * pair + i) * 160 + wop * 80 + 80]
        shared["t2a"] = t2a.astype(F8E4)
        shared["t2b"] = np.ascontiguousarray(t2[:, 640:800]).astype(F8E4)
        # f1d[p, pair*256 + i*128 + f] = f1[p, (2*pair+i)*120 + f] (f<120; pad 8 zeros)
        f1d = np.zeros((80, 512), np.float32)
        for pair in range(2):
            for i in range(2):
                f1d[:, pair * 256 + i * 128 : pair * 256 + i * 128 + 120] = \
                    f1[:, (2 * pair + i) * 120 : (2 * pair + i) * 120 + 120]
        shared["f1d"] = f1d.astype(F8E4)
        f14 = np.zeros((80, 128), np.float32)
        f14[:, 0:120] = f1[:, 480:600]
        shared["f14"] = f14.astype(F8E4)
    else:
        shared["t2"] = t2.astype(BF16)
        shared["f1"] = f1.astype(BF16)

    xh = xr.astype(np.float16)
    if lo == "dr8":
        s = np.float32(2.0 ** LOSC)
        si = np.float32(2.0 ** -LOSC)
        r1 = xr - xh.astype(np.float32)
        l1 = (r1 * s).astype(F8E4)
        r2 = r1 - l1.astype(np.float32) * si
        l2 = (r2 * s).astype(F8E4)
        # xlo per 2-row tile k: [l1 rows 2k,2k+1 | l2 rows 2k,2k+1]
        l1t = l1.reshape(NCORES, 96, 16, 2 * N)
        l2t = l2.reshape(NCORES, 96, 16, 2 * N)
        xlo = np.stack([l1t, l2t], axis=3).reshape(NCORES, 96, 64 * N)
        # t1lo[p, (kh*2+par)*192 + i*96 + m] = t1[p, kh*168+par*84+m] * 2^-LOSC
        t1f = _build_t1(w1)
        t1lo = np.zeros((96, 1920), np.float32)
        for kh in range(5):
            for par in range(2):
                src = t1f[:, kh * 168 + par * 84 : kh * 168 + par * 84 + 84] * si
                for i in range(2):
                    t1lo[:, (kh * 2 + par) * 192 + i * 96 : (kh * 2 + par) * 192 + i * 96 + 84] = src
        shared["t1lo"] = t1lo.astype(ml_dtypes.float8_e5m2)
        in_maps = [
            dict(shared, xh=np.ascontiguousarray(xh[i]), xlo=np.ascontiguousarray(xlo[i]))
            for i in range(NCORES)
        ]
    else:
        xl = (xr - xh.astype(np.float32)).astype(np.float16)
        in_maps = [
            dict(shared, xh=np.ascontiguousarray(xh[i]), xl=np.ascontiguousarray(xl[i]))
            for i in range(NCORES)
        ]

    res = bass_utils.run_bass_kernel_spmd(
        nc, in_maps, core_ids=list(range(NCORES)),
        trace=bool(int(os.environ.get("KERNEL_TRACE", "0"))),
    )
    if res.exec_time_ns is not None:
        print(f"HW exec time: {res.exec_time_ns} ns")
    out = np.stack([r["out"] for r in res.results])  # [8, 10, N]
    return np.ascontiguousarray(out.transpose(0, 2, 1)).reshape(B, 10).astype(np.float32)


# revision 35
# speedup vs baseline: 1.4397x; 1.1954x over previous
"""BinaryLeNet5 forward on 8 TRN2 NeuronCores, pure data parallel (1024 imgs/core).

Mapping summary (per core):
  conv1: kh-accumulated banded-Toeplitz matmuls. lhsT T1[kh] = [96=(c,wi), 168=(mc,wop,wo2,o)]
         built host-side from binarized w1; rhs = direct row-slices of transposed input
         xt[(c,w), (h,n)]. conv1 touches real-valued x, and the net is chaotic in the
         conv1 signs (even 2^-17 rounding flips enough signs to fail the 2e-2 gate), so
         the rhs is an fp16 hi+lo split (residual 2^-22 -> measured 0 final error),
         two accumulating passes per kh at 1 cycle/row each (fp32 native is 4 cyc/row).
  pool+sign: maxpool ho-pairs are the two contiguous halves of each PSUM tile and
         wo-pairs are the two wo-parity PSUM tiles -> 3 contiguous DVE tensor_max ops
         (strided reduce_max was ~1.8x slower), then ACT Sign with f32 per-partition
         bias (bias exact; commutes with max).
  conv2: inputs/weights exactly +-1 -> fp8e4 exact; kh pairs {0,1},{2,3} run as
         DoubleRow matmuls (2 fp8 weights/PE cell, K_eff=168, ~2x column rate), kh4 as
         a plain fp8 matmul. f32 PSUM accumulation of +-1 sums is exact.
  fc1/fc2/fc3: +-1 bf16 matmuls, f32 PSUM, biases applied in f32 via ACT bias operand.
  hardtanh drops out everywhere: sign(clip(x)) == sign(x), max(clip) == clip(max).

DMA: two HW DGE issue queues (sync=hi tiles, scalar=lo tiles+weights); input split
into 2-h-row tiles so the first matmul waits on ~0.55MB, not the full 12.6MB.

Output written as [10, 1024] per core, transposed/stacked on host.
"""

import os
import sys

import numpy as np

sys.path.insert(0, "/opt/trn_rl_repo")

import ml_dtypes  # noqa: E402

BF16 = ml_dtypes.bfloat16
F8E4 = ml_dtypes.float8_e4m3

B = 8192
NCORES = 8
N = B // NCORES  # 1024 images per core
NBLK = 2  # n blocks of 512 columns
NB = N // NBLK  # 512

CONV2_DR = int(os.environ.get("CONV2_DR", "1"))  # fp8 DoubleRow conv2
# conv1 lo-pass mode: "f16" (default) = single fp16 lo pass (exact). "dr8" =
# two fp8 lo-terms in one DoubleRow matmul (descale 2^-14 via e5m2 weights) —
# measured SAME speed as f16 (DR packs K, it does not double the column rate)
# with worse (though passing) accuracy: rel err 0.0111. Kept for reference.
CONV1_LO = os.environ.get("CONV1_LO", "f16")
LOSC = 14  # lo-term scale exponent


def _binarize(w):
    return np.where(w >= 0, 1.0, -1.0).astype(np.float32)


def _build_t1(w1):
    # t1[c*32+wi, kh*168 + par*84 + wo2*6 + o] = w1b[o,c,kh,kw]
    #   wo = 2*wo2 + par (par = wo parity), kw = wi - wo, valid 0<=kw<5
    # Chunk A (par=0) = all even wo, chunk B = all odd wo -> the 2x2 maxpool's
    # wo-pair max is a same-partition tensor_max of the two chunk results.
    w1b = _binarize(w1)  # [6,3,5,5]
    t1 = np.zeros((96, 5 * 168), np.float32)
    for kh in range(5):
        for par in range(2):
            for wo2 in range(14):
                wo = 2 * wo2 + par
                for o in range(6):
                    col = kh * 168 + par * 84 + wo2 * 6 + o
                    for c in range(3):
                        for kw in range(5):
                            wi = wo + kw
                            if wi < 32:
                                t1[c * 32 + wi, col] = w1b[o, c, kh, kw]
    return t1


def _build_t2(w2):
    # t2[w2*6+c, kh*160 + wop*80 + wo2*16 + o] = w2b[o,c,kh,kw], kw = w2-(2*wo2+wop)
    w2b = _binarize(w2)  # [16,6,5,5]
    t2 = np.zeros((84, 5 * 160), np.float32)
    for kh in range(5):
        for wop in range(2):
            for wo2 in range(5):
                wo = 2 * wo2 + wop
                for o in range(16):
                    col = kh * 160 + wop * 80 + wo2 * 16 + o
                    for c in range(6):
                        for kw in range(5):
                            w2i = wo + kw
                            if w2i < 14:
                                t2[w2i * 6 + c, col] = w2b[o, c, kh, kw]
    return t2


def _build_f1(wf1):
    # f1[w*16+o, h*120+f] = wf1b[f, o*25+h*5+w]
    wf1b = _binarize(wf1)  # [120, 400]
    f1 = np.zeros((80, 5 * 120), np.float32)
    for h in range(5):
        for w in range(5):
            for o in range(16):
                f1[w * 16 + o, h * 120 : (h + 1) * 120] = wf1b[:, o * 25 + h * 5 + w]
    return f1


_CACHE = {}


def _get_nc(dr, lo):
    key = f"nc_dr{dr}_{lo}"
    if key in _CACHE:
        return _CACHE[key]
    import concourse.bacc as bacc
    import concourse.mybir as mybir
    import concourse.tile as tile

    f32 = mybir.dt.float32
    f16 = mybir.dt.float16
    bf16 = mybir.dt.bfloat16
    f8e4 = mybir.dt.float8e4
    f8e5 = mybir.dt.float8e5
    x2dt = f8e4 if dr else bf16

    nc = bacc.Bacc()
    if lo == "pack8":
        # A: rows 0-95 = fp16 hi (c,wi), rows 96-127 = fp16 lo of c0.
        # C: rows 0-31 = lo c1 @h, 32-63 = lo c2 @h, 64-95 = lo c1 @h+1,
        #    96-127 = lo c2 @h+1 (kh-pairs packed into the spare K rows).
        xa_d = nc.dram_tensor("xa", [128, 32 * N], f16, kind="ExternalInput")
        xc_d = nc.dram_tensor("xc", [128, 32 * N], f16, kind="ExternalInput")
        t1a_d = nc.dram_tensor("t1a", [128, 840], f16, kind="ExternalInput")
        t1c_d = nc.dram_tensor("t1c", [128, 504], f16, kind="ExternalInput")
    else:
        xh_d = nc.dram_tensor("xh", [96, 32 * N], f16, kind="ExternalInput")
    if lo == "dr8":
        # per 2-row tile k: [l1 row0 | l1 row1 | l2 row0 | l2 row1], N cols each
        xlo_d = nc.dram_tensor("xlo", [96, 64 * N], f8e4, kind="ExternalInput")
        # per (kh,par): [i0: 96 cols | i1: 96 cols], both = t1 slice * 2^-LOSC,
        # M padded 84->96 (DR weight step must be %16==0)
        t1lo_d = nc.dram_tensor("t1lo", [96, 1920], f8e5, kind="ExternalInput")
    elif lo == "f16":
        xl_d = nc.dram_tensor("xl", [96, 32 * N], f16, kind="ExternalInput")
    if lo != "pack8":
        t1_d = nc.dram_tensor("t1", [96, 840], f16, kind="ExternalInput")
    if dr:
        t2a_d = nc.dram_tensor("t2a", [84, 640], f8e4, kind="ExternalInput")
        t2b_d = nc.dram_tensor("t2b", [84, 160], f8e4, kind="ExternalInput")
        f1d_d = nc.dram_tensor("f1d", [80, 512], f8e4, kind="ExternalInput")
        f14_d = nc.dram_tensor("f14", [80, 128], f8e4, kind="ExternalInput")
    else:
        t2_d = nc.dram_tensor("t2", [84, 800], bf16, kind="ExternalInput")
        f1_d = nc.dram_tensor("f1", [80, 600], bf16, kind="ExternalInput")
    f2_d = nc.dram_tensor("f2", [120, 84], bf16, kind="ExternalInput")
    f3_d = nc.dram_tensor("f3", [84, 10], bf16, kind="ExternalInput")
    b1_d = nc.dram_tensor("b1v", [84, 1], f32, kind="ExternalInput")
    b2_d = nc.dram_tensor("b2v", [80, 1], f32, kind="ExternalInput")
    bf1_d = nc.dram_tensor("bf1v", [120, 1], f32, kind="ExternalInput")
    bf2_d = nc.dram_tensor("bf2v", [84, 1], f32, kind="ExternalInput")
    bf3_d = nc.dram_tensor("bf3v", [10, 1], f32, kind="ExternalInput")
    out_d = nc.dram_tensor("out", [10, N], f32, kind="ExternalOutput")

    HCH = 2  # h rows per xt sbuf tile
    NT = 32 // HCH  # 16 tiles per hi/lo
    with tile.TileContext(nc) as tc:
        with (
            tc.tile_pool(name="xtp", bufs=1) as xtp,
            tc.tile_pool(name="wts", bufs=1) as wts,
            tc.tile_pool(name="acts", bufs=1) as acts,
            tc.tile_pool(name="ev", bufs=3) as ev,
            tc.tile_pool(name="ps", bufs=4, space="PSUM") as ps,
        ):
            # ---- DMA plan: sync queue = t1 + hi tiles (conv1 critical path);
            # scalar queue = first lo tiles, then the conv2/fc weights+biases
            # (needed from ~30us on), then the remaining lo tiles. ----
            if lo == "pack8":
                t1s = wts.tile([128, 840], f16, tag="t1")
                nc.scalar.dma_start(out=t1s, in_=t1a_d[:, :])
                t1cs = wts.tile([128, 504], f16, tag="t1c")
                nc.scalar.dma_start(out=t1cs, in_=t1c_d[:, :])
            else:
                t1s = wts.tile([96, 840], f16, tag="t1")
                nc.scalar.dma_start(out=t1s, in_=t1_d[:, :])
            if lo == "dr8":
                t1los = wts.tile([96, 1920], f8e5, tag="t1lo")
                nc.scalar.dma_start(out=t1los, in_=t1lo_d[:, :])

            LCH = 2 * HCH * N if lo == "dr8" else HCH * N  # lo cols per tile
            lodt = f8e4 if lo == "dr8" else f16
            hrows = 128 if lo == "pack8" else 96
            hid = xa_d if lo == "pack8" else xh_d
            if lo == "pack8":
                lod = xc_d
            elif lo == "dr8":
                lod = xlo_d
            else:
                lod = xl_d
            xhs, xls = [], []
            for k in range(NT):
                th = xtp.tile([hrows, HCH * N], f16, tag=f"xh{k}", name=f"xh{k}")
                nc.sync.dma_start(out=th, in_=hid[:, k * HCH * N : (k + 1) * HCH * N])
                xhs.append(th)
            for k in range(3):
                tl = xtp.tile([hrows, LCH], lodt, tag=f"xl{k}", name=f"xl{k}")
                nc.scalar.dma_start(out=tl, in_=lod[:, k * LCH : (k + 1) * LCH])
                xls.append(tl)

            if dr:
                t2as = wts.tile([84, 640], f8e4, tag="t2a")
                nc.scalar.dma_start(out=t2as, in_=t2a_d[:, :])
                t2bs = wts.tile([84, 160], f8e4, tag="t2b")
                nc.scalar.dma_start(out=t2bs, in_=t2b_d[:, :])
                f1ds = wts.tile([80, 512], f8e4, tag="f1d")
                nc.scalar.dma_start(out=f1ds, in_=f1d_d[:, :])
                f14s = wts.tile([80, 128], f8e4, tag="f14")
                nc.scalar.dma_start(out=f14s, in_=f14_d[:, :])
            else:
                t2s = wts.tile([84, 800], bf16, tag="t2")
                nc.scalar.dma_start(out=t2s, in_=t2_d[:, :])
                f1s = wts.tile([80, 600], bf16, tag="f1")
                nc.scalar.dma_start(out=f1s, in_=f1_d[:, :])
            f2s = wts.tile([120, 84], bf16, tag="f2")
            nc.scalar.dma_start(out=f2s, in_=f2_d[:, :])
            f3s = wts.tile([84, 10], bf16, tag="f3")
            nc.scalar.dma_start(out=f3s, in_=f3_d[:, :])
            b1s = wts.tile([84, 1], f32, tag="b1")
            nc.scalar.dma_start(out=b1s, in_=b1_d[:, :])
            b2s = wts.tile([80, 1], f32, tag="b2")
            nc.scalar.dma_start(out=b2s, in_=b2_d[:, :])
            bf1s = wts.tile([120, 1], f32, tag="bf1")
            nc.scalar.dma_start(out=bf1s, in_=bf1_d[:, :])
            bf2s = wts.tile([84, 1], f32, tag="bf2")
            nc.scalar.dma_start(out=bf2s, in_=bf2_d[:, :])
            bf3s = wts.tile([10, 1], f32, tag="bf3")
            nc.scalar.dma_start(out=bf3s, in_=bf3_d[:, :])

            for k in range(3, NT):
                tl = xtp.tile([hrows, LCH], lodt, tag=f"xl{k}", name=f"xl{k}")
                nc.scalar.dma_start(out=tl, in_=lod[:, k * LCH : (k + 1) * LCH])
                xls.append(tl)

            def xhi(h, nb):  # hi rhs [96, NB] for input row h
                off = (h % HCH) * N + nb * NB
                return xhs[h // HCH][:, off : off + NB]

            def xlo(h, nb):
                if lo == "dr8":  # [96, 2, NB]: (l1, l2) pair for DoubleRow
                    t = xls[h // HCH].rearrange("p (i c) -> p i c", i=2)
                    off = (h % HCH) * N + nb * NB
                    return t[:, :, off : off + NB]
                off = (h % HCH) * N + nb * NB
                return xls[h // HCH][:, off : off + NB]

            # One consumer-engine 'touch' per DMA'd bias tile: the touch op
            # carries the DMA wait, so later ops on that engine need no extra
            # wait slot (TRN2 engine instructions have a single wait slot).
            tb1 = wts.tile([84, 1], f32, tag="tb1")
            nc.scalar.copy(tb1, b1s)
            tb2 = wts.tile([80, 1], f32, tag="tb2")
            nc.scalar.copy(tb2, b2s)
            tb3 = wts.tile([120, 1], f32, tag="tb3")
            nc.scalar.copy(tb3, bf1s)
            tb4 = wts.tile([84, 1], f32, tag="tb4")
            nc.scalar.copy(tb4, bf2s)
            tb5 = wts.tile([10, 1], f32, tag="tb5")
            nc.vector.tensor_copy(tb5, bf3s)

            x2 = acts.tile([84, 14 * N], x2dt, tag="x2")
            x3 = acts.tile([80, 5 * N], f8e4 if dr else bf16, tag="x3")
            x4 = acts.tile([120, N], bf16, tag="x4")
            x5 = acts.tile([84, N], bf16, tag="x5")
            outs = acts.tile([10, N], f32, tag="outs")

            # ---- conv1 + pool + sign -> x2 ----
            # psum tile [84, 1024] = (hop 2) x (n 512) blocks; ho-pair pooled as
            # tensor_max of the tile's two contiguous halves; wo-pair = tensor_max
            # of the two parity tiles (same partitions). No strided DVE reads.
            PM = 96 if lo == "dr8" else 84  # psum partitions (DR pads M 84->96)
            for ho2 in range(14):
                for nb in range(NBLK):
                    p = [ps.tile([PM, 2 * NB], f32, tag="ps", name="p1") for _ in range(2)]
                    for par in range(2):
                        if lo == "pack8":
                            # 5 A-streams: hi all-channels + lo-c0, kh = 0..4
                            for kh in range(5):
                                lhs = t1s[:, kh * 168 + par * 84 : kh * 168 + par * 84 + 84]
                                for hop in range(2):
                                    h = 2 * ho2 + hop + kh
                                    nc.tensor.matmul(
                                        p[par][:, hop * NB : hop * NB + NB],
                                        lhs, xhi(h, nb),
                                        start=(kh == 0), stop=False,
                                    )
                            # 3 C-streams: lo c1/c2 kh-pairs {0,1},{2,3},{4,-}
                            for j in range(3):
                                lhs = t1cs[:, j * 168 + par * 84 : j * 168 + par * 84 + 84]
                                for hop in range(2):
                                    h = 2 * ho2 + hop + 2 * j
                                    nc.tensor.matmul(
                                        p[par][:, hop * NB : hop * NB + NB],
                                        lhs, xlo(h, nb),
                                        start=False, stop=(j == 2),
                                    )
                            continue
                        if lo == "dr8":
                            # Phase 1: all 10 DoubleRow lo matmuls (same PE mode,
                            # weights switch only per kh — keeps LDW pipelined;
                            # interleaving DR and normal MMs cost ~250 cyc/pair).
                            # The 96-row kh0 DR opens the accumulation region.
                            for kh in range(5):
                                lod_lhs = t1los[
                                    :, (kh * 2 + par) * 192 : (kh * 2 + par) * 192 + 192
                                ].rearrange("p (i m) -> p i m", i=2)
                                for hop in range(2):
                                    h = 2 * ho2 + hop + kh
                                    nc.tensor.matmul(
                                        p[par][:, hop * NB : hop * NB + NB],
                                        lod_lhs, xlo(h, nb),
                                        start=(kh == 0), stop=False,
                                        perf_mode=mybir.MatmulPerfMode.DoubleRow,
                                    )
                            # Phase 2: all 10 fp16 hi matmuls accumulate on top.
                            for kh in range(5):
                                lhs = t1s[:, kh * 168 + par * 84 : kh * 168 + par * 84 + 84]
                                for hop in range(2):
                                    h = 2 * ho2 + hop + kh
                                    nc.tensor.matmul(
                                        p[par][0:84, hop * NB : hop * NB + NB],
                                        lhs, xhi(h, nb),
                                        start=False, stop=(kh == 4),
                                        skip_group_check=True,
                                    )
                        else:
                            for kh in range(5):
                                lhs = t1s[:, kh * 168 + par * 84 : kh * 168 + par * 84 + 84]
                                for hop in range(2):
                                    h = 2 * ho2 + hop + kh
                                    out84 = p[par][0:84, hop * NB : hop * NB + NB]
                                    nc.tensor.matmul(
                                        out84, lhs, xhi(h, nb),
                                        start=(kh == 0), stop=False,
                                    )
                                    nc.tensor.matmul(
                                        out84, lhs, xlo(h, nb),
                                        start=False, stop=(kh == 4),
                                    )
                    e1 = []
                    for par in range(2):
                        e = ev.tile([96, NB], f32, tag="ea", name="e1")[0:84]
                        nc.vector.reduce_max(e, p[par][0:84, :].rearrange("q (h n) -> q n h", h=2), axis=mybir.AxisListType.X)
                        e1.append(e)
                    e2 = ev.tile([96, NB], f32, tag="ec", name="e2")[0:84]
                    nc.vector.tensor_max(e2, e1[0], e1[1])
                    nc.scalar.sign(
                        x2[:, ho2 * N + nb * NB : ho2 * N + nb * NB + NB],
                        e2, bias=b1s,
                    )
            # ---- conv2 + pool + sign -> x3 ----
            x2r = x2.rearrange("p (h n) -> p h n", h=14)
            for ho2 in range(5):
                for nb in range(NBLK):
                    p2 = [ps.tile([80, 2 * NB], f32, tag="ps", name="p2") for _ in range(2)]
                    for wop in range(2):
                        if dr:
                            for pair in range(2):  # kh {0,1} and {2,3}
                                lhs = t2as[
                                    :, pair * 320 + wop * 160 : pair * 320 + wop * 160 + 160
                                ].rearrange("p (i m) -> p i m", i=2)
                                for hop in range(2):
                                    h0 = 2 * ho2 + hop + 2 * pair
                                    nc.tensor.matmul(
                                        p2[wop][:, hop * NB : hop * NB + NB], lhs,
                                        x2r[:, h0 : h0 + 2, nb * NB : nb * NB + NB],
                                        start=(pair == 0), stop=False,
                                        perf_mode=mybir.MatmulPerfMode.DoubleRow,
                                    )
                            lhs4 = t2bs[:, wop * 80 : wop * 80 + 80]
                            for hop in range(2):
                                h_in = 2 * ho2 + hop + 4
                                nc.tensor.matmul(
                                    p2[wop][:, hop * NB : hop * NB + NB], lhs4,
                                    x2[:, h_in * N + nb * NB : h_in * N + nb * NB + NB],
                                    start=False, stop=True,
                                )
                        else:
                            for kh in range(5):
                                lhs = t2s[:, kh * 160 + wop * 80 : kh * 160 + wop * 80 + 80]
                                for hop in range(2):
                                    h_in = 2 * ho2 + hop + kh
                                    nc.tensor.matmul(
                                        p2[wop][:, hop * NB : hop * NB + NB], lhs,
                                        x2[:, h_in * N + nb * NB : h_in * N + nb * NB + NB],
                                        start=(kh == 0), stop=(kh == 4),
                                    )
                    ew = []
                    for wop in range(2):
                        # conv2 PSUM values are exact small integers -> bf16 pooled
                        # tiles are exact and 16-bit DVE ops run at 2x rate.
                        e = ev.tile([96, NB], bf16, tag="eb", name="e3")[0:80]
                        nc.vector.reduce_max(e, p2[wop].rearrange("q (h n) -> q n h", h=2), axis=mybir.AxisListType.X)
                        ew.append(e)
                    e4 = ev.tile([96, NB], bf16, tag="ed", name="e4")[0:80]
                    nc.vector.tensor_max(e4, ew[0], ew[1])
                    nc.scalar.sign(
                        x3[:, ho2 * N + nb * NB : ho2 * N + nb * NB + NB], e4, bias=b2s
                    )

            # ---- fc1 -> fc2 -> fc3 ----
            # 4 n-sub-blocks of 256 with stage-interleaving: shortens the exposed
            # serial MM->sign->MM chain at the kernel tail; fc1 h5-pairs {0,1},{2,3}
            # run as fp8 DoubleRow (M padded 120->128), h5=4 as a plain fp8 matmul.
            NBF = 2
            NF = N // NBF  # 512
            x3r = x3.rearrange("p (h n) -> p h n", h=5)
            p3 = [
                ps.tile([128 if dr else 120, NF], f32, tag="ps", name="p3")
                for _ in range(NBF)
            ]
            for nb in range(NBF):
                if dr:
                    for pair in range(2):
                        lhs = f1ds[:, pair * 256 : pair * 256 + 256].rearrange(
                            "p (i m) -> p i m", i=2
                        )
                        nc.tensor.matmul(
                            p3[nb], lhs,
                            x3r[:, 2 * pair : 2 * pair + 2, nb * NF : nb * NF + NF],
                            start=(pair == 0), stop=False,
                            perf_mode=mybir.MatmulPerfMode.DoubleRow,
                        )
                    nc.tensor.matmul(
                        p3[nb], f14s,
                        x3[:, 4 * N + nb * NF : 4 * N + nb * NF + NF],
                        start=False, stop=True,
                    )
                else:
                    for h5 in range(5):
                        nc.tensor.matmul(
                            p3[nb], f1s[:, h5 * 120 : (h5 + 1) * 120],
                            x3[:, h5 * N + nb * NF : h5 * N + nb * NF + NF],
                            start=(h5 == 0), stop=(h5 == 4),
                        )
            for nb in range(NBF):
                nc.scalar.sign(x4[:, nb * NF : nb * NF + NF], p3[nb][0:120], bias=bf1s)
            p4 = [ps.tile([84, NF], f32, tag="ps", name="p4") for _ in range(NBF)]
            for nb in range(NBF):
                nc.tensor.matmul(p4[nb], f2s, x4[:, nb * NF : nb * NF + NF], start=True, stop=True)
            for nb in range(NBF):
                nc.scalar.sign(x5[:, nb * NF : nb * NF + NF], p4[nb], bias=bf2s)
            p5 = [ps.tile([10, NF], f32, tag="ps", name="p5") for _ in range(NBF)]
            for nb in range(NBF):
                nc.tensor.matmul(p5[nb], f3s, x5[:, nb * NF : nb * NF + NF], start=True, stop=True)
            for nb in range(NBF):
                nc.vector.tensor_scalar_add(outs[:, nb * NF : nb * NF + NF], p5[nb], bf3s)
                nc.sync.dma_start(
                    out=out_d[:, nb * NF : nb * NF + NF],
                    in_=outs[:, nb * NF : nb * NF + NF],
                )

    nc.finalize()
    _CACHE[key] = nc
    return nc


def _install_ntff_hook():
    """The container's antenv stub lacks axon_hooks; synthesize it and register
    the ctypes-based NTFF profile hook from the axon boot module."""
    if "hook" in _CACHE:
        return
    _CACHE["hook"] = True
    try:
        import types
        import antenv

        if not hasattr(antenv, "axon_hooks"):
            store = {"h": None}
            m = types.ModuleType("antenv.axon_hooks")
            m.set_axon_ntff_profile_hook = lambda h: store.update(h=h)
            m.get_axon_ntff_profile_hook = lambda: store["h"]
            sys.modules["antenv.axon_hooks"] = m
            antenv.axon_hooks = m
            sys.path.insert(0, "/root/.axon_site")
            from trn_agent_boot.trn_boot import _ntff_profile_via_ctypes

            m.set_axon_ntff_profile_hook(
                _ntff_profile_via_ctypes("/opt/axon/libaxon_pjrt.so")
            )
    except Exception as e:  # profiling is best-effort
        print(f"ntff hook install failed: {e}", file=sys.stderr)


def kernel(x, w1, b1, w2, b2, wf1, bf1, wf2, bf2, wf3, bf3):
    dr = CONV2_DR
    lo = CONV1_LO
    nc = _get_nc(dr, lo)
    _install_ntff_hook()
    from concourse import bass_utils

    # host-side relayout: xt[core][c*32+w, h*N+n] = x[core*N+n, c, h, w]
    xr = np.ascontiguousarray(
        x.reshape(NCORES, N, 3, 32, 32).transpose(0, 2, 4, 3, 1)
    ).reshape(NCORES, 96, 32 * N)

    t1 = _build_t1(w1).astype(np.float16)
    t2 = _build_t2(w2)  # [84, 800] f32, col = kh*160 + wop*80 + m
    f1 = _build_f1(wf1)  # [80, 600] f32, col = h5*120 + f
    f2 = np.ascontiguousarray(_binarize(wf2).T).astype(BF16)  # [120, 84]
    f3 = np.ascontiguousarray(_binarize(wf3).T).astype(BF16)  # [84, 10]
    b1v = np.tile(b1.astype(np.float32), 14).reshape(84, 1)  # part = wo2*6+o
    b2v = np.tile(b2.astype(np.float32), 5).reshape(80, 1)  # part = wo2*16+o
    shared = {
        "t1": t1, "f2": f2, "f3": f3,
        "b1v": b1v, "b2v": b2v,
        "bf1v": bf1.astype(np.float32).reshape(120, 1),
        "bf2v": bf2.astype(np.float32).reshape(84, 1),
        "bf3v": bf3.astype(np.float32).reshape(10, 1),
    }
    if dr:
        # t2a[p, pair*320 + wop*160 + i*80 + m] = t2[p, (2*pair+i)*160 + wop*80 + m]
        t2a = np.zeros((84, 640), np.float32)
        for pair in range(2):
            for wop in range(2):
                for i in range(2):
                    t2a[:, pair * 320 + wop * 160 + i * 80 : pair * 320 + wop * 160 + i * 80 + 80] = \
                        t2[:, (2 * pair + i) * 160 + wop * 80 : (2 * pair + i) * 160 + wop * 80 + 80]
        shared["t2a"] = t2a.astype(F8E4)
        shared["t2b"] = np.ascontiguousarray(t2[:, 640:800]).astype(F8E4)
        # f1d[p, pair*256 + i*128 + f] = f1[p, (2*pair+i)*120 + f] (f<120; pad 8 zeros)
        f1d = np.zeros((80, 512), np.float32)
        for pair in range(2):
            for i in range(2):
                f1d[:, pair * 256 + i * 128 : pair * 256 + i * 128 + 120] = \
                    f1[:, (2 * pair + i) * 120 : (2 * pair + i) * 120 + 120]
        shared["f1d"] = f1d.astype(F8E4)
        f14 = np.zeros((80, 128), np.float32)
        f14[:, 0:120] = f1[:, 480:600]
        shared["f14"] = f14.astype(F8E4)
    else:
        shared["t2"] = t2.astype(BF16)
        shared["f1"] = f1.astype(BF16)

    xh = xr.astype(np.float16)
    if lo == "pack8":
        xl = (xr - xh.astype(np.float32)).astype(np.float16)
        xa = np.concatenate([xh, xl[:, 0:32]], axis=1)  # [NC, 128, 32N]
        xl4 = xl.reshape(NCORES, 96, 32, N)
        l1 = xl4[:, 32:64]
        l2 = xl4[:, 64:96]
        l1s = np.zeros_like(l1)
        l1s[:, :, :31] = l1[:, :, 1:]
        l2s = np.zeros_like(l2)
        l2s[:, :, :31] = l2[:, :, 1:]
        xc = np.concatenate([l1, l2, l1s, l2s], axis=1).reshape(NCORES, 128, 32 * N)
        t1a = np.concatenate([t1, t1[0:32]], axis=0)  # [128, 840]
        t1c = np.zeros((128, 504), np.float16)
        for j in range(3):
            t1c[0:32, j * 168 : (j + 1) * 168] = t1[32:64, 2 * j * 168 : (2 * j + 1) * 168]
            t1c[32:64, j * 168 : (j + 1) * 168] = t1[64:96, 2 * j * 168 : (2 * j + 1) * 168]
            if 2 * j + 1 < 5:
                t1c[64:96, j * 168 : (j + 1) * 168] = t1[32:64, (2 * j + 1) * 168 : (2 * j + 2) * 168]
                t1c[96:128, j * 168 : (j + 1) * 168] = t1[64:96, (2 * j + 1) * 168 : (2 * j + 2) * 168]
        del shared["t1"]
        shared["t1a"] = t1a
        shared["t1c"] = t1c
        in_maps = [
            dict(shared, xa=np.ascontiguousarray(xa[i]), xc=np.ascontiguousarray(xc[i]))
            for i in range(NCORES)
        ]
    elif lo == "dr8":
        s = np.float32(2.0 ** LOSC)
        si = np.float32(2.0 ** -LOSC)
        r1 = xr - xh.astype(np.float32)
        l1 = (r1 * s).astype(F8E4)
        r2 = r1 - l1.astype(np.float32) * si
        l2 = (r2 * s).astype(F8E4)
        # xlo per 2-row tile k: [l1 rows 2k,2k+1 | l2 rows 2k,2k+1]
        l1t = l1.reshape(NCORES, 96, 16, 2 * N)
        l2t = l2.reshape(NCORES, 96, 16, 2 * N)
        xlo = np.stack([l1t, l2t], axis=3).reshape(NCORES, 96, 64 * N)
        # t1lo[p, (kh*2+par)*192 + i*96 + m] = t1[p, kh*168+par*84+m] * 2^-LOSC
        t1f = _build_t1(w1)
        t1lo = np.zeros((96, 1920), np.float32)
        for kh in range(5):
            for par in range(2):
                src = t1f[:, kh * 168 + par * 84 : kh * 168 + par * 84 + 84] * si
                for i in range(2):
                    t1lo[:, (kh * 2 + par) * 192 + i * 96 : (kh * 2 + par) * 192 + i * 96 + 84] = src
        shared["t1lo"] = t1lo.astype(ml_dtypes.float8_e5m2)
        in_maps = [
            dict(shared, xh=np.ascontiguousarray(xh[i]), xlo=np.ascontiguousarray(xlo[i]))
            for i in range(NCORES)
        ]
    else:
        xl = (xr - xh.astype(np.float32)).astype(np.float16)
        in_maps = [
            dict(shared, xh=np.ascontiguousarray(xh[i]), xl=np.ascontiguousarray(xl[i]))
            for i in range(NCORES)
        ]

    res = bass_utils.run_bass_kernel_spmd(
        nc, in_maps, core_ids=list(range(NCORES)),
        trace=bool(int(os.environ.get("KERNEL_TRACE", "0"))),
    )
    if res.exec_time_ns is not None:
        print(f"HW exec time: {res.exec_time_ns} ns")
    out = np.stack([r["out"] for r in res.results])  # [8, 10, N]
    return np.ascontiguousarray(out.transpose(0, 2, 1)).reshape(B, 10).astype(np.float32)


# revision 36
# speedup vs baseline: 1.4426x; 1.0020x over previous
"""BinaryLeNet5 forward on 8 TRN2 NeuronCores, pure data parallel (1024 imgs/core).

Mapping summary (per core):
  conv1: kh-accumulated banded-Toeplitz matmuls. lhsT T1[kh] = [96=(c,wi), 168=(mc,wop,wo2,o)]
         built host-side from binarized w1; rhs = direct row-slices of transposed input
         xt[(c,w), (h,n)]. conv1 touches real-valued x, and the net is chaotic in the
         conv1 signs (even 2^-17 rounding flips enough signs to fail the 2e-2 gate), so
         the rhs is an fp16 hi+lo split (residual 2^-22 -> measured 0 final error),
         two accumulating passes per kh at 1 cycle/row each (fp32 native is 4 cyc/row).
  pool+sign: maxpool ho-pairs are the two contiguous halves of each PSUM tile and
         wo-pairs are the two wo-parity PSUM tiles -> 3 contiguous DVE tensor_max ops
         (strided reduce_max was ~1.8x slower), then ACT Sign with f32 per-partition
         bias (bias exact; commutes with max).
  conv2: inputs/weights exactly +-1 -> fp8e4 exact; kh pairs {0,1},{2,3} run as
         DoubleRow matmuls (2 fp8 weights/PE cell, K_eff=168, ~2x column rate), kh4 as
         a plain fp8 matmul. f32 PSUM accumulation of +-1 sums is exact.
  fc1/fc2/fc3: +-1 bf16 matmuls, f32 PSUM, biases applied in f32 via ACT bias operand.
  hardtanh drops out everywhere: sign(clip(x)) == sign(x), max(clip) == clip(max).

DMA: two HW DGE issue queues (sync=hi tiles, scalar=lo tiles+weights); input split
into 2-h-row tiles so the first matmul waits on ~0.55MB, not the full 12.6MB.

Output written as [10, 1024] per core, transposed/stacked on host.
"""

import os
import sys

import numpy as np

sys.path.insert(0, "/opt/trn_rl_repo")

import ml_dtypes  # noqa: E402

BF16 = ml_dtypes.bfloat16
F8E4 = ml_dtypes.float8_e4m3

B = 8192
NCORES = 8
N = B // NCORES  # 1024 images per core
NBLK = 2  # n blocks of 512 columns
NB = N // NBLK  # 512

CONV2_DR = int(os.environ.get("CONV2_DR", "1"))  # fp8 DoubleRow conv2
# conv1 lo-pass mode (all exact-precision fp16 except dr8):
#   "pack8" (default): 8 streams/position — conv1's K=96 leaves 32 spare PE rows;
#     layout A packs lo-c0 under the hi streams (5 MMs), layout C packs lo-c1/c2
#     kh-PAIRS into K=128 (3 MMs). 20% fewer conv1 cycles than f16, rel err 0.
#   "f16": 10 streams (5 hi + 5 lo). "dr8": lo as fp8 DoubleRow pair — same speed
#     as f16 (DR packs K, not column rate), rel err 0.0111. Kept for reference.
CONV1_LO = os.environ.get("CONV1_LO", "pack8")
LOSC = 14  # lo-term scale exponent


def _binarize(w):
    return np.where(w >= 0, 1.0, -1.0).astype(np.float32)


def _build_t1(w1):
    # t1[c*32+wi, kh*168 + par*84 + wo2*6 + o] = w1b[o,c,kh,kw]
    #   wo = 2*wo2 + par (par = wo parity), kw = wi - wo, valid 0<=kw<5
    # Chunk A (par=0) = all even wo, chunk B = all odd wo -> the 2x2 maxpool's
    # wo-pair max is a same-partition tensor_max of the two chunk results.
    w1b = _binarize(w1)  # [6,3,5,5]
    t1 = np.zeros((96, 5 * 168), np.float32)
    for kh in range(5):
        for par in range(2):
            for wo2 in range(14):
                wo = 2 * wo2 + par
                for o in range(6):
                    col = kh * 168 + par * 84 + wo2 * 6 + o
                    for c in range(3):
                        for kw in range(5):
                            wi = wo + kw
                            if wi < 32:
                                t1[c * 32 + wi, col] = w1b[o, c, kh, kw]
    return t1


def _build_t2(w2):
    # t2[w2*6+c, kh*160 + wop*80 + wo2*16 + o] = w2b[o,c,kh,kw], kw = w2-(2*wo2+wop)
    w2b = _binarize(w2)  # [16,6,5,5]
    t2 = np.zeros((84, 5 * 160), np.float32)
    for kh in range(5):
        for wop in range(2):
            for wo2 in range(5):
                wo = 2 * wo2 + wop
                for o in range(16):
                    col = kh * 160 + wop * 80 + wo2 * 16 + o
                    for c in range(6):
                        for kw in range(5):
                            w2i = wo + kw
                            if w2i < 14:
                                t2[w2i * 6 + c, col] = w2b[o, c, kh, kw]
    return t2


def _build_f1(wf1):
    # f1[w*16+o, h*120+f] = wf1b[f, o*25+h*5+w]
    wf1b = _binarize(wf1)  # [120, 400]
    f1 = np.zeros((80, 5 * 120), np.float32)
    for h in range(5):
        for w in range(5):
            for o in range(16):
                f1[w * 16 + o, h * 120 : (h + 1) * 120] = wf1b[:, o * 25 + h * 5 + w]
    return f1


_CACHE = {}


def _get_nc(dr, lo):
    key = f"nc_dr{dr}_{lo}"
    if key in _CACHE:
        return _CACHE[key]
    import concourse.bacc as bacc
    import concourse.mybir as mybir
    import concourse.tile as tile

    f32 = mybir.dt.float32
    f16 = mybir.dt.float16
    bf16 = mybir.dt.bfloat16
    f8e4 = mybir.dt.float8e4
    f8e5 = mybir.dt.float8e5
    x2dt = f8e4 if dr else bf16

    nc = bacc.Bacc()
    if lo == "pack8":
        # A: rows 0-95 = fp16 hi (c,wi), rows 96-127 = fp16 lo of c0.
        # C: rows 0-31 = lo c1 @h, 32-63 = lo c2 @h, 64-95 = lo c1 @h+1,
        #    96-127 = lo c2 @h+1 (kh-pairs packed into the spare K rows).
        xa_d = nc.dram_tensor("xa", [128, 32 * N], f16, kind="ExternalInput")
        xc_d = nc.dram_tensor("xc", [128, 32 * N], f16, kind="ExternalInput")
        t1a_d = nc.dram_tensor("t1a", [128, 840], f16, kind="ExternalInput")
        t1c_d = nc.dram_tensor("t1c", [128, 504], f16, kind="ExternalInput")
    else:
        xh_d = nc.dram_tensor("xh", [96, 32 * N], f16, kind="ExternalInput")
    if lo == "dr8":
        # per 2-row tile k: [l1 row0 | l1 row1 | l2 row0 | l2 row1], N cols each
        xlo_d = nc.dram_tensor("xlo", [96, 64 * N], f8e4, kind="ExternalInput")
        # per (kh,par): [i0: 96 cols | i1: 96 cols], both = t1 slice * 2^-LOSC,
        # M padded 84->96 (DR weight step must be %16==0)
        t1lo_d = nc.dram_tensor("t1lo", [96, 1920], f8e5, kind="ExternalInput")
    elif lo == "f16":
        xl_d = nc.dram_tensor("xl", [96, 32 * N], f16, kind="ExternalInput")
    if lo != "pack8":
        t1_d = nc.dram_tensor("t1", [96, 840], f16, kind="ExternalInput")
    if dr:
        t2a_d = nc.dram_tensor("t2a", [84, 640], f8e4, kind="ExternalInput")
        t2b_d = nc.dram_tensor("t2b", [84, 160], f8e4, kind="ExternalInput")
        f1d_d = nc.dram_tensor("f1d", [80, 512], f8e4, kind="ExternalInput")
        f14_d = nc.dram_tensor("f14", [80, 128], f8e4, kind="ExternalInput")
    else:
        t2_d = nc.dram_tensor("t2", [84, 800], bf16, kind="ExternalInput")
        f1_d = nc.dram_tensor("f1", [80, 600], bf16, kind="ExternalInput")
    f2_d = nc.dram_tensor("f2", [120, 84], bf16, kind="ExternalInput")
    f3_d = nc.dram_tensor("f3", [84, 10], bf16, kind="ExternalInput")
    b1_d = nc.dram_tensor("b1v", [84, 1], f32, kind="ExternalInput")
    b2_d = nc.dram_tensor("b2v", [80, 1], f32, kind="ExternalInput")
    bf1_d = nc.dram_tensor("bf1v", [120, 1], f32, kind="ExternalInput")
    bf2_d = nc.dram_tensor("bf2v", [84, 1], f32, kind="ExternalInput")
    bf3_d = nc.dram_tensor("bf3v", [10, 1], f32, kind="ExternalInput")
    out_d = nc.dram_tensor("out", [10, N], f32, kind="ExternalOutput")

    HCH = 2  # h rows per xt sbuf tile
    NT = 32 // HCH  # 16 tiles per hi/lo
    with tile.TileContext(nc) as tc:
        with (
            tc.tile_pool(name="xtp", bufs=1) as xtp,
            tc.tile_pool(name="wts", bufs=1) as wts,
            tc.tile_pool(name="acts", bufs=1) as acts,
            tc.tile_pool(name="ev", bufs=3) as ev,
            tc.tile_pool(name="ps", bufs=4, space="PSUM") as ps,
        ):
            # ---- DMA plan: sync queue = t1 + hi tiles (conv1 critical path);
            # scalar queue = first lo tiles, then the conv2/fc weights+biases
            # (needed from ~30us on), then the remaining lo tiles. ----
            if lo == "pack8":
                t1s = wts.tile([128, 840], f16, tag="t1")
                nc.scalar.dma_start(out=t1s, in_=t1a_d[:, :])
                t1cs = wts.tile([128, 504], f16, tag="t1c")
                nc.scalar.dma_start(out=t1cs, in_=t1c_d[:, :])
            else:
                t1s = wts.tile([96, 840], f16, tag="t1")
                nc.scalar.dma_start(out=t1s, in_=t1_d[:, :])
            if lo == "dr8":
                t1los = wts.tile([96, 1920], f8e5, tag="t1lo")
                nc.scalar.dma_start(out=t1los, in_=t1lo_d[:, :])

            LCH = 2 * HCH * N if lo == "dr8" else HCH * N  # lo cols per tile
            lodt = f8e4 if lo == "dr8" else f16
            hrows = 128 if lo == "pack8" else 96
            hid = xa_d if lo == "pack8" else xh_d
            if lo == "pack8":
                lod = xc_d
            elif lo == "dr8":
                lod = xlo_d
            else:
                lod = xl_d
            xhs, xls = [], []
            for k in range(NT):
                th = xtp.tile([hrows, HCH * N], f16, tag=f"xh{k}", name=f"xh{k}")
                nc.sync.dma_start(out=th, in_=hid[:, k * HCH * N : (k + 1) * HCH * N])
                xhs.append(th)
            for k in range(3):
                tl = xtp.tile([hrows, LCH], lodt, tag=f"xl{k}", name=f"xl{k}")
                nc.scalar.dma_start(out=tl, in_=lod[:, k * LCH : (k + 1) * LCH])
                xls.append(tl)

            if dr:
                t2as = wts.tile([84, 640], f8e4, tag="t2a")
                nc.scalar.dma_start(out=t2as, in_=t2a_d[:, :])
                t2bs = wts.tile([84, 160], f8e4, tag="t2b")
                nc.scalar.dma_start(out=t2bs, in_=t2b_d[:, :])
                f1ds = wts.tile([80, 512], f8e4, tag="f1d")
                nc.scalar.dma_start(out=f1ds, in_=f1d_d[:, :])
                f14s = wts.tile([80, 128], f8e4, tag="f14")
                nc.scalar.dma_start(out=f14s, in_=f14_d[:, :])
            else:
                t2s = wts.tile([84, 800], bf16, tag="t2")
                nc.scalar.dma_start(out=t2s, in_=t2_d[:, :])
                f1s = wts.tile([80, 600], bf16, tag="f1")
                nc.scalar.dma_start(out=f1s, in_=f1_d[:, :])
            f2s = wts.tile([120, 84], bf16, tag="f2")
            nc.scalar.dma_start(out=f2s, in_=f2_d[:, :])
            f3s = wts.tile([84, 10], bf16, tag="f3")
            nc.scalar.dma_start(out=f3s, in_=f3_d[:, :])
            b1s = wts.tile([84, 1], f32, tag="b1")
            nc.scalar.dma_start(out=b1s, in_=b1_d[:, :])
            b2s = wts.tile([80, 1], f32, tag="b2")
            nc.scalar.dma_start(out=b2s, in_=b2_d[:, :])
            bf1s = wts.tile([120, 1], f32, tag="bf1")
            nc.scalar.dma_start(out=bf1s, in_=bf1_d[:, :])
            bf2s = wts.tile([84, 1], f32, tag="bf2")
            nc.scalar.dma_start(out=bf2s, in_=bf2_d[:, :])
            bf3s = wts.tile([10, 1], f32, tag="bf3")
            nc.scalar.dma_start(out=bf3s, in_=bf3_d[:, :])

            for k in range(3, NT):
                tl = xtp.tile([hrows, LCH], lodt, tag=f"xl{k}", name=f"xl{k}")
                nc.scalar.dma_start(out=tl, in_=lod[:, k * LCH : (k + 1) * LCH])
                xls.append(tl)

            def xhi(h, nb):  # hi rhs [96, NB] for input row h
                off = (h % HCH) * N + nb * NB
                return xhs[h // HCH][:, off : off + NB]

            def xlo(h, nb):
                if lo == "dr8":  # [96, 2, NB]: (l1, l2) pair for DoubleRow
                    t = xls[h // HCH].rearrange("p (i c) -> p i c", i=2)
                    off = (h % HCH) * N + nb * NB
                    return t[:, :, off : off + NB]
                off = (h % HCH) * N + nb * NB
                return xls[h // HCH][:, off : off + NB]

            # One consumer-engine 'touch' per DMA'd bias tile: the touch op
            # carries the DMA wait, so later ops on that engine need no extra
            # wait slot (TRN2 engine instructions have a single wait slot).
            tb1 = wts.tile([84, 1], f32, tag="tb1")
            nc.scalar.copy(tb1, b1s)
            tb2 = wts.tile([80, 1], f32, tag="tb2")
            nc.scalar.copy(tb2, b2s)
            tb3 = wts.tile([120, 1], f32, tag="tb3")
            nc.scalar.copy(tb3, bf1s)
            tb4 = wts.tile([84, 1], f32, tag="tb4")
            nc.scalar.copy(tb4, bf2s)
            tb5 = wts.tile([10, 1], f32, tag="tb5")
            nc.vector.tensor_copy(tb5, bf3s)

            x2 = acts.tile([84, 14 * N], x2dt, tag="x2")
            x3 = acts.tile([80, 5 * N], f8e4 if dr else bf16, tag="x3")
            x4 = acts.tile([120, N], bf16, tag="x4")
            x5 = acts.tile([84, N], bf16, tag="x5")
            outs = acts.tile([10, N], f32, tag="outs")

            # ---- conv1 + pool + sign -> x2 ----
            # psum tile [84, 1024] = (hop 2) x (n 512) blocks; ho-pair pooled as
            # tensor_max of the tile's two contiguous halves; wo-pair = tensor_max
            # of the two parity tiles (same partitions). No strided DVE reads.
            PM = 96 if lo == "dr8" else 84  # psum partitions (DR pads M 84->96)
            for ho2 in range(14):
                for nb in range(NBLK):
                    p = [ps.tile([PM, 2 * NB], f32, tag="ps", name="p1") for _ in range(2)]
                    for par in range(2):
                        if lo == "pack8":
                            # 5 A-streams: hi all-channels + lo-c0, kh = 0..4
                            for kh in range(5):
                                lhs = t1s[:, kh * 168 + par * 84 : kh * 168 + par * 84 + 84]
                                for hop in range(2):
                                    h = 2 * ho2 + hop + kh
                                    nc.tensor.matmul(
                                        p[par][:, hop * NB : hop * NB + NB],
                                        lhs, xhi(h, nb),
                                        start=(kh == 0), stop=False,
                                    )
                            # 3 C-streams: lo c1/c2 kh-pairs {0,1},{2,3},{4,-}
                            for j in range(3):
                                lhs = t1cs[:, j * 168 + par * 84 : j * 168 + par * 84 + 84]
                                for hop in range(2):
                                    h = 2 * ho2 + hop + 2 * j
                                    nc.tensor.matmul(
                                        p[par][:, hop * NB : hop * NB + NB],
                                        lhs, xlo(h, nb),
                                        start=False, stop=(j == 2),
                                    )
                            continue
                        if lo == "dr8":
                            # Phase 1: all 10 DoubleRow lo matmuls (same PE mode,
                            # weights switch only per kh — keeps LDW pipelined;
                            # interleaving DR and normal MMs cost ~250 cyc/pair).
                            # The 96-row kh0 DR opens the accumulation region.
                            for kh in range(5):
                                lod_lhs = t1los[
                                    :, (kh * 2 + par) * 192 : (kh * 2 + par) * 192 + 192
                                ].rearrange("p (i m) -> p i m", i=2)
                                for hop in range(2):
                                    h = 2 * ho2 + hop + kh
                                    nc.tensor.matmul(
                                        p[par][:, hop * NB : hop * NB + NB],
                                        lod_lhs, xlo(h, nb),
                                        start=(kh == 0), stop=False,
                                        perf_mode=mybir.MatmulPerfMode.DoubleRow,
                                    )
                            # Phase 2: all 10 fp16 hi matmuls accumulate on top.
                            for kh in range(5):
                                lhs = t1s[:, kh * 168 + par * 84 : kh * 168 + par * 84 + 84]
                                for hop in range(2):
                                    h = 2 * ho2 + hop + kh
                                    nc.tensor.matmul(
                                        p[par][0:84, hop * NB : hop * NB + NB],
                                        lhs, xhi(h, nb),
                                        start=False, stop=(kh == 4),
                                        skip_group_check=True,
                                    )
                        else:
                            for kh in range(5):
                                lhs = t1s[:, kh * 168 + par * 84 : kh * 168 + par * 84 + 84]
                                for hop in range(2):
                                    h = 2 * ho2 + hop + kh
                                    out84 = p[par][0:84, hop * NB : hop * NB + NB]
                                    nc.tensor.matmul(
                                        out84, lhs, xhi(h, nb),
                                        start=(kh == 0), stop=False,
                                    )
                                    nc.tensor.matmul(
                                        out84, lhs, xlo(h, nb),
                                        start=False, stop=(kh == 4),
                                    )
                    e1 = []
                    for par in range(2):
                        e = ev.tile([96, NB], f32, tag="ea", name="e1")[0:84]
                        nc.vector.reduce_max(e, p[par][0:84, :].rearrange("q (h n) -> q n h", h=2), axis=mybir.AxisListType.X)
                        e1.append(e)
                    e2 = ev.tile([96, NB], f32, tag="ec", name="e2")[0:84]
                    nc.vector.tensor_max(e2, e1[0], e1[1])
                    nc.scalar.sign(
                        x2[:, ho2 * N + nb * NB : ho2 * N + nb * NB + NB],
                        e2, bias=b1s,
                    )
            # ---- conv2 + pool + sign -> x3 ----
            x2r = x2.rearrange("p (h n) -> p h n", h=14)
            for ho2 in range(5):
                for nb in range(NBLK):
                    p2 = [ps.tile([80, 2 * NB], f32, tag="ps", name="p2") for _ in range(2)]
                    for wop in range(2):
                        if dr:
                            for pair in range(2):  # kh {0,1} and {2,3}
                                lhs = t2as[
                                    :, pair * 320 + wop * 160 : pair * 320 + wop * 160 + 160
                                ].rearrange("p (i m) -> p i m", i=2)
                                for hop in range(2):
                                    h0 = 2 * ho2 + hop + 2 * pair
                                    nc.tensor.matmul(
                                        p2[wop][:, hop * NB : hop * NB + NB], lhs,
                                        x2r[:, h0 : h0 + 2, nb * NB : nb * NB + NB],
                                        start=(pair == 0), stop=False,
                                        perf_mode=mybir.MatmulPerfMode.DoubleRow,
                                    )
                            lhs4 = t2bs[:, wop * 80 : wop * 80 + 80]
                            for hop in range(2):
                                h_in = 2 * ho2 + hop + 4
                                nc.tensor.matmul(
                                    p2[wop][:, hop * NB : hop * NB + NB], lhs4,
                                    x2[:, h_in * N + nb * NB : h_in * N + nb * NB + NB],
                                    start=False, stop=True,
                                )
                        else:
                            for kh in range(5):
                                lhs = t2s[:, kh * 160 + wop * 80 : kh * 160 + wop * 80 + 80]
                                for hop in range(2):
                                    h_in = 2 * ho2 + hop + kh
                                    nc.tensor.matmul(
                                        p2[wop][:, hop * NB : hop * NB + NB], lhs,
                                        x2[:, h_in * N + nb * NB : h_in * N + nb * NB + NB],
                                        start=(kh == 0), stop=(kh == 4),
                                    )
                    ew = []
                    for wop in range(2):
                        # conv2 PSUM values are exact small integers -> bf16 pooled
                        # tiles are exact and 16-bit DVE ops run at 2x rate.
                        e = ev.tile([96, NB], bf16, tag="eb", name="e3")[0:80]
                        nc.vector.reduce_max(e, p2[wop].rearrange("q (h n) -> q n h", h=2), axis=mybir.AxisListType.X)
                        ew.append(e)
                    e4 = ev.tile([96, NB], bf16, tag="ed", name="e4")[0:80]
                    nc.vector.tensor_max(e4, ew[0], ew[1])
                    nc.scalar.sign(
                        x3[:, ho2 * N + nb * NB : ho2 * N + nb * NB + NB], e4, bias=b2s
                    )

            # ---- fc1 -> fc2 -> fc3 ----
            # 4 n-sub-blocks of 256 with stage-interleaving: shortens the exposed
            # serial MM->sign->MM chain at the kernel tail; fc1 h5-pairs {0,1},{2,3}
            # run as fp8 DoubleRow (M padded 120->128), h5=4 as a plain fp8 matmul.
            NBF = 2
            NF = N // NBF  # 512
            x3r = x3.rearrange("p (h n) -> p h n", h=5)
            p3 = [
                ps.tile([128 if dr else 120, NF], f32, tag="ps", name="p3")
                for _ in range(NBF)
            ]
            for nb in range(NBF):
                if dr:
                    for pair in range(2):
                        lhs = f1ds[:, pair * 256 : pair * 256 + 256].rearrange(
                            "p (i m) -> p i m", i=2
                        )
                        nc.tensor.matmul(
                            p3[nb], lhs,
                            x3r[:, 2 * pair : 2 * pair + 2, nb * NF : nb * NF + NF],
                            start=(pair == 0), stop=False,
                            perf_mode=mybir.MatmulPerfMode.DoubleRow,
                        )
                    nc.tensor.matmul(
                        p3[nb], f14s,
                        x3[:, 4 * N + nb * NF : 4 * N + nb * NF + NF],
                        start=False, stop=True,
                    )
                else:
                    for h5 in range(5):
                        nc.tensor.matmul(
                            p3[nb], f1s[:, h5 * 120 : (h5 + 1) * 120],
                            x3[:, h5 * N + nb * NF : h5 * N + nb * NF + NF],
                            start=(h5 == 0), stop=(h5 == 4),
                        )
            for nb in range(NBF):
                nc.scalar.sign(x4[:, nb * NF : nb * NF + NF], p3[nb][0:120], bias=bf1s)
            p4 = [ps.tile([84, NF], f32, tag="ps", name="p4") for _ in range(NBF)]
            for nb in range(NBF):
                nc.tensor.matmul(p4[nb], f2s, x4[:, nb * NF : nb * NF + NF], start=True, stop=True)
            for nb in range(NBF):
                nc.scalar.sign(x5[:, nb * NF : nb * NF + NF], p4[nb], bias=bf2s)
            p5 = [ps.tile([10, NF], f32, tag="ps", name="p5") for _ in range(NBF)]
            for nb in range(NBF):
                nc.tensor.matmul(p5[nb], f3s, x5[:, nb * NF : nb * NF + NF], start=True, stop=True)
            for nb in range(NBF):
                nc.vector.tensor_scalar_add(outs[:, nb * NF : nb * NF + NF], p5[nb], bf3s)
                nc.sync.dma_start(
                    out=out_d[:, nb * NF : nb * NF + NF],
                    in_=outs[:, nb * NF : nb * NF + NF],
                )

    nc.finalize()
    _CACHE[key] = nc
    return nc


def _install_ntff_hook():
    """The container's antenv stub lacks axon_hooks; synthesize it and register
    the ctypes-based NTFF profile hook from the axon boot module."""
    if "hook" in _CACHE:
        return
    _CACHE["hook"] = True
    try:
        import types
        import antenv

        if not hasattr(antenv, "axon_hooks"):
            store = {"h": None}
            m = types.ModuleType("antenv.axon_hooks")
            m.set_axon_ntff_profile_hook = lambda h: store.update(h=h)
            m.get_axon_ntff_profile_hook = lambda: store["h"]
            sys.modules["antenv.axon_hooks"] = m
            antenv.axon_hooks = m
            sys.path.insert(0, "/root/.axon_site")
            from trn_agent_boot.trn_boot import _ntff_profile_via_ctypes

            m.set_axon_ntff_profile_hook(
                _ntff_profile_via_ctypes("/opt/axon/libaxon_pjrt.so")
            )
    except Exception as e:  # profiling is best-effort
        print(f"ntff hook install failed: {e}", file=sys.stderr)


def kernel(x, w1, b1, w2, b2, wf1, bf1, wf2, bf2, wf3, bf3):
    dr = CONV2_DR
    lo = CONV1_LO
    nc = _get_nc(dr, lo)
    _install_ntff_hook()
    from concourse import bass_utils

    # host-side relayout: xt[core][c*32+w, h*N+n] = x[core*N+n, c, h, w]
    xr = np.ascontiguousarray(
        x.reshape(NCORES, N, 3, 32, 32).transpose(0, 2, 4, 3, 1)
    ).reshape(NCORES, 96, 32 * N)

    t1 = _build_t1(w1).astype(np.float16)
    t2 = _build_t2(w2)  # [84, 800] f32, col = kh*160 + wop*80 + m
    f1 = _build_f1(wf1)  # [80, 600] f32, col = h5*120 + f
    f2 = np.ascontiguousarray(_binarize(wf2).T).astype(BF16)  # [120, 84]
    f3 = np.ascontiguousarray(_binarize(wf3).T).astype(BF16)  # [84, 10]
    b1v = np.tile(b1.astype(np.float32), 14).reshape(84, 1)  # part = wo2*6+o
    b2v = np.tile(b2.astype(np.float32), 5).reshape(80, 1)  # part = wo2*16+o
    shared = {
        "t1": t1, "f2": f2, "f3": f3,
        "b1v": b1v, "b2v": b2v,
        "bf1v": bf1.astype(np.float32).reshape(120, 1),
        "bf2v": bf2.astype(np.float32).reshape(84, 1),
        "bf3v": bf3.astype(np.float32).reshape(10, 1),
    }
    if dr:
        # t2a[p, pair*320 + wop*160 + i*80 + m] = t2[p, (2*pair+i)*160 + wop*80 + m]
        t2a = np.zeros((84, 640), np.float32)
        for pair in range(2):
            for wop in range(2):
                for i in range(2):
                    t2a[:, pair * 320 + wop * 160 + i * 80 : pair * 320 + wop * 160 + i * 80 + 80] = \
                        t2[:, (2 * pair + i) * 160 + wop * 80 : (2 * pair + i) * 160 + wop * 80 + 80]
        shared["t2a"] = t2a.astype(F8E4)
        shared["t2b"] = np.ascontiguousarray(t2[:, 640:800]).astype(F8E4)
        # f1d[p, pair*256 + i*128 + f] = f1[p, (2*pair+i)*120 + f] (f<120; pad 8 zeros)
        f1d = np.zeros((80, 512), np.float32)
        for pair in range(2):
            for i in range(2):
                f1d[:, pair * 256 + i * 128 : pair * 256 + i * 128 + 120] = \
                    f1[:, (2 * pair + i) * 120 : (2 * pair + i) * 120 + 120]
        shared["f1d"] = f1d.astype(F8E4)
        f14 = np.zeros((80, 128), np.float32)
        f14[:, 0:120] = f1[:, 480:600]
        shared["f14"] = f14.astype(F8E4)
    else:
        shared["t2"] = t2.astype(BF16)
        shared["f1"] = f1.astype(BF16)

    xh = xr.astype(np.float16)
    if lo == "pack8":
        xl = (xr - xh.astype(np.float32)).astype(np.float16)
        xa = np.concatenate([xh, xl[:, 0:32]], axis=1)  # [NC, 128, 32N]
        xl4 = xl.reshape(NCORES, 96, 32, N)
        l1 = xl4[:, 32:64]
        l2 = xl4[:, 64:96]
        l1s = np.zeros_like(l1)
        l1s[:, :, :31] = l1[:, :, 1:]
        l2s = np.zeros_like(l2)
        l2s[:, :, :31] = l2[:, :, 1:]
        xc = np.concatenate([l1, l2, l1s, l2s], axis=1).reshape(NCORES, 128, 32 * N)
        t1a = np.concatenate([t1, t1[0:32]], axis=0)  # [128, 840]
        t1c = np.zeros((128, 504), np.float16)
        for j in range(3):
            t1c[0:32, j * 168 : (j + 1) * 168] = t1[32:64, 2 * j * 168 : (2 * j + 1) * 168]
            t1c[32:64, j * 168 : (j + 1) * 168] = t1[64:96, 2 * j * 168 : (2 * j + 1) * 168]
            if 2 * j + 1 < 5:
                t1c[64:96, j * 168 : (j + 1) * 168] = t1[32:64, (2 * j + 1) * 168 : (2 * j + 2) * 168]
                t1c[96:128, j * 168 : (j + 1) * 168] = t1[64:96, (2 * j + 1) * 168 : (2 * j + 2) * 168]
        del shared["t1"]
        shared["t1a"] = t1a
        shared["t1c"] = t1c
        in_maps = [
            dict(shared, xa=np.ascontiguousarray(xa[i]), xc=np.ascontiguousarray(xc[i]))
            for i in range(NCORES)
        ]
    elif lo == "dr8":
        s = np.float32(2.0 ** LOSC)
        si = np.float32(2.0 ** -LOSC)
        r1 = xr - xh.astype(np.float32)
        l1 = (r1 * s).astype(F8E4)
        r2 = r1 - l1.astype(np.float32) * si
        l2 = (r2 * s).astype(F8E4)
        # xlo per 2-row tile k: [l1 rows 2k,2k+1 | l2 rows 2k,2k+1]
        l1t = l1.reshape(NCORES, 96, 16, 2 * N)
        l2t = l2.reshape(NCORES, 96, 16, 2 * N)
        xlo = np.stack([l1t, l2t], axis=3).reshape(NCORES, 96, 64 * N)
        # t1lo[p, (kh*2+par)*192 + i*96 + m] = t1[p, kh*168+par*84+m] * 2^-LOSC
        t1f = _build_t1(w1)
        t1lo = np.zeros((96, 1920), np.float32)
        for kh in range(5):
            for par in range(2):
                src = t1f[:, kh * 168 + par * 84 : kh * 168 + par * 84 + 84] * si
                for i in range(2):
                    t1lo[:, (kh * 2 + par) * 192 + i * 96 : (kh * 2 + par) * 192 + i * 96 + 84] = src
        shared["t1lo"] = t1lo.astype(ml_dtypes.float8_e5m2)
        in_maps = [
            dict(shared, xh=np.ascontiguousarray(xh[i]), xlo=np.ascontiguousarray(xlo[i]))
            for i in range(NCORES)
        ]
    else:
        xl = (xr - xh.astype(np.float32)).astype(np.float16)
        in_maps = [
            dict(shared, xh=np.ascontiguousarray(xh[i]), xl=np.ascontiguousarray(xl[i]))
            for i in range(NCORES)
        ]

    res = bass_utils.run_bass_kernel_spmd(
        nc, in_maps, core_ids=list(range(NCORES)),
        trace=bool(int(os.environ.get("KERNEL_TRACE", "0"))),
    )
    if res.exec_time_ns is not None:
        print(f"HW exec time: {res.exec_time_ns} ns")
    out = np.stack([r["out"] for r in res.results])  # [8, 10, N]
    return np.ascontiguousarray(out.transpose(0, 2, 1)).reshape(B, 10).astype(np.float32)
